# revision 63
# baseline (speedup 1.0000x reference)
"""Expert-parallel MoE layer for Trainium2 (Bass/Tile, 8 NeuronCores).

Strategy (hardcoded for B=4, T=2048, C=1024, E=8, H=2728, top_k=2):
  - Expert-parallel: core e owns expert e's weights (w1/w2/w3[e]).
  - Host computes the router (top-2 ids AND the softmax combine weights in
    exact fp32) and performs the all-to-all token dispatch/combine as the
    shard/unshard step. The per-token gate weight rides in as a small fp32
    vector, pre-scaled by the fp8 scale factors.
  - Each core computes the full expert FFN y = (silu(x@w1.T) * (x@w3.T))
    @ w2.T * g for its tokens, entirely in fp8-e4m3 DoubleRow matmuls
    (K=256 per instruction, 0.5 cycles/column — 2x the fp32r MAC rate).

Precision: every GEMM uses a 3-term hi/lo split, all at one shared scale so
the three products accumulate in a single PSUM chain:
    A@W ~= Ah@Wh + Al@Wh + Ah@Wl      (drops only the Al@Wl term, ~7e-4)
with Ah = e4m3(A*S), Al = e4m3(A*S - Ah). End-to-end rel err ~2e-3 vs the
2e-2 gate. x/w splits happen on host; the phase-A output s is split on
device (ACT copy for the hi part, DVE subtract for the residual).

Layouts are pre-arranged on host so every matmul operand is a direct SBUF
slice: stationary tiles [128, 2, 128] (DoubleRow K-pair x M), moving tiles
[128, 2, tw]. s_hi stays resident in SBUF; s_lo spills to DRAM and streams
back during phase B (bandwidth is far under the PE time either way).
"""

import os
import sys
from contextlib import ExitStack

import numpy as np
import ml_dtypes

for _p in ("/opt/trn_rl_repo", "/root/.axon_site/_ro/trn_rl_repo"):
    if os.path.isdir(_p) and _p not in sys.path:
        sys.path.insert(0, _p)

import concourse.mybir as mybir
import concourse.tile as tile
from concourse.tile_rust import add_dep_helper
from concourse import bacc
from concourse.bass_utils import run_bass_kernel_spmd

FP32 = mybir.dt.float32
FP8 = mybir.dt.float8e4
ALU = mybir.AluOpType
AF = mybir.ActivationFunctionType
DR = mybir.MatmulPerfMode.DoubleRow
E4NP = ml_dtypes.float8_e4m3

E = 8            # experts == cores
C = 1024         # model dim
H0 = 2728        # ffn hidden dim
NKC = C // 256   # 4 DoubleRow contraction tiles over C
KH = 22          # 128-row h tiles (padded H)
NKH = KH // 2    # 11 DoubleRow contraction tiles over padded H
HP = KH * 128    # 2816
KC8 = C // 128   # 8 output c tiles
TT = 512         # max token tile (fp32 PSUM bank = 512 floats)
CAP_MAX = 2304   # per-launch token cap (SBUF budget); split into runs beyond

# fp8 scale factors. All hi/lo parts share their tensor's scale so the three
# split products accumulate in one PSUM chain.
SX = 16.0        # x*16: |x|<5.1 -> <82, x_lo ~0.4 (normal range)
SW = 1024.0      # w*1024: |w|<0.11 -> <113
SH = 8.0         # s*8: |s|<12 -> <96 (clip-safe), s_lo ~0.07
SA = SX * SW     # phase-A psum scale
SB = SH * SW     # phase-B psum scale

_CACHE = {}
_WCACHE = {}
LAST_RESULTS = None

# startup-schedule knobs (fixed by a TimelineSim sweep)
XH_SCALAR = (1, 3)      # xh tile indices that ride the ACT queue
W0SPLIT = True          # split whi[0] into w1/w3 halves around xh0
RAMP = (128, 256, 256)  # leading token-tile widths


def _token_tiles(cap):
    # sub-512 tiles go FIRST (ascending): the first matmuls' DMA
    # dependencies are smaller, so the PE starts (and ramps) earlier. A
    # 128-wide leader is fine at fp8-DR (no narrow-tile rate penalty).
    ramp = list(RAMP)
    while sum(ramp) > max(0, cap - 256) and len(ramp) > 1:
        ramp.pop()
    widths = list(ramp)
    left = cap - sum(ramp)
    if left % TT:
        widths.append(left % TT)
    widths += [TT] * (left // TT)
    widths.sort()
    tiles = []
    off = 0
    for w in widths:
        tiles.append((off, w))
        off += w
    return tiles


def _split8(a):
    """a is pre-scaled fp32; return (hi, lo) e4m3 arrays at the same scale."""
    hi = np.clip(a, -240.0, 240.0).astype(E4NP)
    lo = (a - hi.astype(np.float32)).astype(E4NP)
    return hi, lo


def _build(cap):
    """Build + compile the SPMD program for `cap` tokens per core."""
    assert cap % 128 == 0
    tiles = _token_tiles(cap)
    last = len(tiles) - 1
    nc = bacc.Bacc("TRN2", target_bir_lowering=False, debug=False, num_devices=E)

    # x packed tile-major: per (partition, token-tile) the [NKC, 2, tw]
    # block is contiguous, so every tile's DMA moves >=2KB chunks (chunks
    # under 512B pay a 2x DMA-bus penalty in HW)
    xh = nc.dram_tensor("xh", [128, NKC * 2 * cap], FP8, kind="ExternalInput").ap()
    xl = nc.dram_tensor("xl", [128, NKC * 2 * cap], FP8, kind="ExternalInput").ap()
    # w1h+w3h (resp. w1l+w3l) fused per h-tile: one DMA instead of two
    # (fixed cost per DMA dominates these small transfers)
    whi = nc.dram_tensor("whi", [KH, 128, 2, NKC, 2, 128], FP8, kind="ExternalInput").ap()
    wlo = nc.dram_tensor("wlo", [KH, 128, 2, NKC, 2, 128], FP8, kind="ExternalInput").ap()
    w2h = nc.dram_tensor("w2h", [KC8, 128, NKH, 2, 128], FP8, kind="ExternalInput").ap()
    w2l = nc.dram_tensor("w2l", [KC8, 128, NKH, 2, 128], FP8, kind="ExternalInput").ap()
    gsc = nc.dram_tensor("gsc", [cap], FP32, kind="ExternalInput").ap()
    yt = nc.dram_tensor("yt", [KC8, 128, cap], FP32, kind="ExternalOutput").ap()

    with tile.TileContext(nc) as tc, ExitStack() as top:
        dramp = top.enter_context(tc.tile_pool(name="dram", bufs=1, space="DRAM"))
        # one scratch tensor per token tile so the phase-B reload of tile t
        # only depends on tile t's writes, not the whole phase A
        ntile = len(tiles)
        slo_dram = [dramp.tile([128, NKH, 2, TT], FP8, tag=f"slo{t}",
                               name=f"slo_dram{t}")
                    for t in range(ntile)]

        shp = top.enter_context(tc.tile_pool(name="sres", bufs=1))
        s_hi = shp.tile([128, NKH, 2, cap], FP8)

        # w2 resident for the whole kernel; loads interleaved into phase A's
        # h-loop so they hide behind compute without starving startup DMA
        w2p = top.enter_context(tc.tile_pool(name="w2res", bufs=1))
        w2h_sb = [w2p.tile([128, NKH, 2, 128], FP8, tag=f"w2h{c}",
                           name=f"w2h_sb{c}") for c in range(KC8)]
        w2l_sb = [w2p.tile([128, NKH, 2, 128], FP8, tag=f"w2l{c}",
                           name=f"w2l_sb{c}") for c in range(KC8)]
        w2_loads = [(w2h_sb[c], w2h[c]) for c in range(KC8)] + \
                   [(w2l_sb[c], w2l[c]) for c in range(KC8)]

        gbc = top.enter_context(tc.tile_pool(name="gbc", bufs=1))
        g_sb = []

        def emit_g():
            # gate-weight rows: tiny loads + partition broadcasts on the
            # SWDGE queue, emitted mid-phase-A where that queue has slack —
            # NOT at the phase boundary, where they'd sit behind the s_lo
            # write backlog and stall the first y-multiplies
            for t, (to, tw) in enumerate(tiles):
                grow = gbc.tile([1, TT], FP32, tag="grow", name=f"grow{t}",
                                bufs=2)
                nc.gpsimd.dma_start(grow[0:1, :tw], gsc[to:to + tw])
                gt = gbc.tile([128, tw], FP32, tag=f"g{t}", name=f"g_sb{t}")
                nc.gpsimd.partition_broadcast(gt[:], grow[0:1, :tw])
                g_sb.append(gt)

        # phase B's PSUM pool is allocated up front so it lands in banks
        # disjoint from phase A's — otherwise B's first chain waits ~1us
        # for A's tail to release a recycled bank
        psb = top.enter_context(tc.tile_pool(name="psB", bufs=3, space="PSUM"))
        anchor = None
        with ExitStack() as pha:
            xp = pha.enter_context(tc.tile_pool(name="xres", bufs=1))
            xh_sb = [xp.tile([128, NKC, 2, tw], FP8, tag=f"xh{t}",
                             name=f"xh_sb{t}") for t, (to, tw) in enumerate(tiles)]
            xl_sb = [xp.tile([128, NKC, 2, tw], FP8, tag=f"xl{t}",
                             name=f"xl_sb{t}") for t, (to, tw) in enumerate(tiles)]
            wst = pha.enter_context(tc.tile_pool(name="wst", bufs=4))

            def walloc(h):
                return (
                    wst.tile([128, 2, NKC, 2, 128], FP8, tag="whi", name=f"whi_{h}"),
                    wst.tile([128, 2, NKC, 2, 128], FP8, tag="wlo", name=f"wlo_{h}"),
                )

            # startup streams split across the two free queues in exact
            # consumption order (per-queue DMA processing is FIFO): SYNC
            # carries the hi parts (consumed first in every chain) + xh;
            # the gpsimd/SWDGE queue carries xl + the fused lo parts. The
            # ACT queue must stay empty here: each DMA on it would occupy
            # the ACT sequencer ~1.3us and push the silu/quantize chain
            # (and with it PSUM recycling) out by that much. x rides ahead
            # of the h>=1 weights: each xh tile is consumed ~1us after the
            # previous, while w[h] only gates the next 11us-long h-sweep.
            def wload_hi(h, wt):
                nc.sync.dma_start(wt[0][:], whi[h])

            def wload_lo(h, wt):
                nc.gpsimd.dma_start(wt[1][:], wlo[h])

            # only h0/h1 preload: the startup DMA wall is the global
            # DMA-engine bandwidth, so deferring h2+ weights (needed only
            # ~22us in) out of the startup window shrinks the stall
            npre = min(2, KH)
            w_cur = {h: walloc(h) for h in range(npre)}
            # h0's hi weights optionally split in two: the first chain only
            # needs the w1 half, so it rides ahead of xh0, w3 follows
            if W0SPLIT:
                nc.sync.dma_start(w_cur[0][0][:, 0], whi[0][:, 0])
            else:
                wload_hi(0, w_cur[0])
            for ti, (to, tw) in enumerate(tiles):
                # some xh tiles ride the ACT queue: a third startup channel
                # (ACT has no compute until the first silu lands, well
                # after these triggers retire)
                xq = nc.scalar if ti in XH_SCALAR else nc.sync
                xq.dma_start(
                    xh_sb[ti][:],
                    xh[:, 8 * to:8 * (to + tw)].rearrange(
                        "p (k i t) -> p k i t", k=NKC, i=2))
                nc.gpsimd.dma_start(
                    xl_sb[ti][:],
                    xl[:, 8 * to:8 * (to + tw)].rearrange(
                        "p (k i t) -> p k i t", k=NKC, i=2))
                if ti == 0:
                    if W0SPLIT:
                        nc.sync.dma_start(w_cur[0][0][:, 1], whi[0][:, 1])
                    wload_lo(0, w_cur[0])
                    wload_hi(1, w_cur[1])
                    wload_lo(1, w_cur[1])

            psa = pha.enter_context(tc.tile_pool(name="psA", bufs=2, space="PSUM"))
            stg = pha.enter_context(tc.tile_pool(name="stg", bufs=3))
            # deep staging: s_lo DMA-out rides the busy SWDGE queue, so the
            # writes may lag the compute by several (h,t) groups
            slop = pha.enter_context(tc.tile_pool(name="slo", bufs=12))

            def emit_ht(h, t, wt):
                nonlocal anchor
                whi_t, wlo_t = wt
                to, tw = tiles[t]
                hd, blk = divmod(h, 2)
                p1 = psa.tile([128, TT], FP32, tag="p1", name=f"p1_{h}_{t}")
                p3 = psa.tile([128, TT], FP32, tag="p3", name=f"p3_{h}_{t}")
                xh_t, xl_t = xh_sb[t], xl_sb[t]
                for pp, wh_, wl_ in ((p1, whi_t[:, 0], wlo_t[:, 0]),
                                     (p3, whi_t[:, 1], wlo_t[:, 1])):
                    n = 0
                    for xs_, ws_ in ((xh_t, wh_), (xl_t, wh_), (xh_t, wl_)):
                        for kd in range(NKC):
                            anchor = nc.tensor.matmul(
                                pp[:, :tw], ws_[:, kd], xs_[:, kd],
                                start=(n == 0), stop=(n == 3 * NKC - 1),
                                perf_mode=DR)
                            n += 1
                sa = stg.tile([128, TT], FP32, tag="sa", name=f"sa{h}_{t}")
                nc.scalar.activation(sa[:, :tw], p1[:, :tw], AF.Silu,
                                     scale=1.0 / SA)
                t1 = stg.tile([128, TT], FP32, tag="t1", name=f"t1_{h}_{t}")
                acc = stg.tile([128, 1], FP32, tag="acc", name=f"acc{h}_{t}")
                nc.vector.affine_mul_reduce(t1[:, :tw], acc[:], p3[:, :tw],
                                            sa[:, :tw], SH / SA, 0.0)
                hi_sl = s_hi[:, hd, blk, to:to + tw]
                nc.scalar.activation(hi_sl, t1[:, :tw], AF.Copy)
                slo = slop.tile([128, TT], FP8, tag="slo", name=f"slo{h}_{t}")
                nc.vector.tensor_tensor(slo[:, :tw], t1[:, :tw], hi_sl,
                                        op=ALU.subtract)
                nc.gpsimd.dma_start(slo_dram[t][:, hd, blk, :tw],
                                    slo[:, :tw])

            # emission: h0/h1 interleaved token-major — each arriving x tile
            # feeds 2x the PE work, halving the startup feed-rate demand on
            # the global DMA engines
            for t in range(len(tiles)):
                for h in range(npre):
                    emit_ht(h, t, w_cur[h])
            for h in range(npre, KH):
                wt = walloc(h)
                wload_hi(h, wt)
                wload_lo(h, wt)
                w_cur[h] = wt
                j = h - npre
                if j < len(w2_loads):
                    dst, src = w2_loads[j]
                    w2dma = nc.gpsimd.dma_start(dst[:], src)
                    add_dep_helper(w2dma.ins, anchor.ins,
                                   reason="delay w2 prefetch")
                if h == 6:
                    emit_g()
                for t in range(len(tiles)):
                    emit_ht(h, t, w_cur[h])
            # two stragglers (KH - npre = 18 slots for 16 w2 loads) — none

        # ---- phase B: y = 3-term(s @ w2.T) * g ----
        with ExitStack() as phb:
            sin = phb.enter_context(tc.tile_pool(name="sin", bufs=2))
            yp = phb.enter_context(tc.tile_pool(name="yst", bufs=4))
            # big tiles in the middle; the smallest tile LAST so the final
            # y writeback (which trails the last matmul) is the shortest
            b_order = sorted(range(len(tiles)),
                             key=lambda i: (-tiles[i][1], i))
            b_order = b_order[:-1] + [b_order[-1]]
            sm = min(range(len(tiles)), key=lambda i: tiles[i][1])
            b_order = [i for i in b_order if i != sm] + [sm]
            # the first tile's s_lo reload rides SYNC (idle through phase A,
            # and the wait on that tile's writes resolves ~10us before the
            # A/B boundary); later tiles go per-tile on the ACT queue, which
            # frees up right at the boundary
            for bi, t in enumerate(b_order):
                to, tw = tiles[t]
                sl_t = sin.tile([128, NKH, 2, TT], FP8, tag="sl",
                                name=f"sl_sb{t}")
                q = nc.sync if bi == 0 else nc.scalar
                q.dma_start(sl_t[:, :, :, :tw], slo_dram[t][:, :, :, :tw])
                for c in range(KC8):
                    py = psb.tile([128, TT], FP32, tag="py", name=f"py{t}_{c}")
                    n = 0
                    # the s_lo group goes LAST so the chain can start before
                    # the reload DMA of this tile's s_lo has landed
                    for s_, w_ in ((s_hi, w2h_sb[c]), (s_hi, w2l_sb[c]),
                                   (None, w2h_sb[c])):
                        for hd in range(NKH):
                            mv = (sl_t[:, hd, :, :tw] if s_ is None
                                  else s_[:, hd, :, to:to + tw])
                            nc.tensor.matmul(py[:, :tw], w_[:, hd], mv,
                                             start=(n == 0),
                                             stop=(n == 3 * NKH - 1),
                                             perf_mode=DR)
                            n += 1
                    yb = yp.tile([128, TT], FP32, tag="y", name=f"yb{t}_{c}")
                    nc.vector.tensor_tensor(yb[:, :tw], py[:, :tw], g_sb[t][:],
                                            op=ALU.mult)
                    # alternate writeback queues: halves the per-queue y
                    # rate so the final transfer doesn't trail the compute
                    yq = nc.sync if c % 2 == 0 else nc.scalar
                    yq.dma_start(yt[c, :, to:to + tw], yb[:, :tw])

    nc.compile()
    return nc


def _prep_weights(gw, w1, w2, w3):
    """Quantize + arrange all per-expert weight tensors (host, cached)."""
    wmaps = []
    for e in range(E):
        m = {}
        his, los = {}, {}
        for nm, w in (("w1", w1[e]), ("w3", w3[e])):
            wt = np.zeros((HP, C), np.float32)
            wt[:H0] = w
            hi, lo = _split8(wt * SW)
            # [HP, C] -> [KH, 128m, NKC, 2, 128p] -> [KH, 128p, NKC, 2, 128m]
            his[nm] = hi.reshape(KH, 128, NKC, 2, 128).transpose(0, 4, 2, 3, 1)
            los[nm] = lo.reshape(KH, 128, NKC, 2, 128).transpose(0, 4, 2, 3, 1)
        m["whi"] = np.ascontiguousarray(
            np.stack([his["w1"], his["w3"]], axis=2))
        m["wlo"] = np.ascontiguousarray(
            np.stack([los["w1"], los["w3"]], axis=2))
        wt = np.zeros((C, HP), np.float32)
        wt[:, :H0] = w2[e]
        hi, lo = _split8(wt * SW)
        # [C, HP] -> [KC8, 128m, NKH, 2, 128p] -> [KC8, 128p, NKH, 2, 128m]
        for part, arr in (("h", hi), ("l", lo)):
            a = arr.reshape(KC8, 128, NKH, 2, 128).transpose(0, 4, 2, 3, 1)
            m["w2" + part] = np.ascontiguousarray(a)
        wmaps.append(m)
    return wmaps


def kernel(x, gate_w, w1, w2, w3, top_k):
    global LAST_RESULTS
    x = np.asarray(x, dtype=np.float32)
    gw = np.asarray(gate_w, dtype=np.float32)
    w1 = np.asarray(w1, dtype=np.float32)
    w2 = np.asarray(w2, dtype=np.float32)
    w3 = np.asarray(w3, dtype=np.float32)
    assert int(np.asarray(top_k)) == 2
    Bb, T, Cc = x.shape
    N = Bb * T
    assert Cc == C and w1.shape == (E, H0, C)

    xf = np.ascontiguousarray(x.reshape(N, C))
    # Router on host (exact fp32): top-2 selection + softmax combine weights.
    logits = xf @ gw.T
    order = np.argsort(-logits, axis=1, kind="stable")[:, :2]
    vals = np.take_along_axis(logits, order, axis=1)
    sw = np.exp(vals - vals.max(axis=1, keepdims=True))
    sw /= sw.sum(axis=1, keepdims=True)
    tok, gtok = [], []
    for e in range(E):
        sel = order == e
        idx = np.nonzero(sel.any(axis=1))[0]
        tok.append(idx)
        gtok.append(sw[sel].astype(np.float32))

    key = (w1.shape, float(w1[0, 0, :8].sum()), float(w2[-1, -1, :8].sum()),
           float(w3[0, -1, :8].sum()))
    wm = _WCACHE.get(key)
    if wm is None:
        wm = _prep_weights(gw, w1, w2, w3)
        _WCACHE.clear()
        _WCACHE[key] = wm

    # quantize x once (full token set), dispatch indexes the fp8 arrays
    xs = xf * SX
    xh_full, xl_full = _split8(xs)

    out = np.zeros((N, C), np.float32)
    nchunk = (max(t.size for t in tok) + CAP_MAX - 1) // CAP_MAX
    for ci in range(nchunk):
        tokc, gc = [], []
        for e in range(E):
            lo_ = (ci * tok[e].size) // nchunk
            hi_ = ((ci + 1) * tok[e].size) // nchunk
            tokc.append(tok[e][lo_:hi_])
            gc.append(gtok[e][lo_:hi_])
        cap = max(TT, ((max(t.size for t in tokc) + 127) // 128) * 128)
        if cap not in _CACHE:
            _CACHE[cap] = _build(cap)
        nc = _CACHE[cap]
        in_maps = []
        for e in range(E):
            idx = tokc[e]
            n = idx.size
            im = dict(wm[e])
            tls = _token_tiles(cap)
            for nm, full in (("xh", xh_full), ("xl", xl_full)):
                xe = np.zeros((cap, C), E4NP)
                xe[:n] = full[idx]
                # [cap, C] -> [128p, NKC, 2, cap] with c = kd*256+i*128+p,
                # then packed tile-major: per partition the [NKC, 2, tw]
                # block of each token tile is contiguous
                a = xe.T.reshape(NKC, 2, 128, cap).transpose(2, 0, 1, 3)
                im[nm] = np.concatenate(
                    [np.ascontiguousarray(a[:, :, :, to:to + tw]).reshape(128, -1)
                     for to, tw in tls], axis=1)
            g = np.zeros(cap, np.float32)
            g[:n] = gc[e] / SB
            im["gsc"] = g
            in_maps.append(im)

        res = run_bass_kernel_spmd(nc, in_maps, core_ids=list(range(E)))
        LAST_RESULTS = res

        for e in range(E):
            idx = tokc[e]
            n = idx.size
            ye = res.results[e]["yt"].reshape(C, cap).T
            out[idx] += ye[:n]
    return out.reshape(Bb, T, C)


# revision 64
# speedup vs baseline: 1.0004x; 1.0004x over previous
"""Expert-parallel MoE layer for Trainium2 (Bass/Tile, 8 NeuronCores).

Strategy (hardcoded for B=4, T=2048, C=1024, E=8, H=2728, top_k=2):
  - Expert-parallel: core e owns expert e's weights (w1/w2/w3[e]).
  - Host computes the router (top-2 ids AND the softmax combine weights in
    exact fp32) and performs the all-to-all token dispatch/combine as the
    shard/unshard step. The per-token gate weight rides in as a small fp32
    vector, pre-scaled by the fp8 scale factors.
  - Each core computes the full expert FFN y = (silu(x@w1.T) * (x@w3.T))
    @ w2.T * g for its tokens, entirely in fp8-e4m3 DoubleRow matmuls
    (K=256 per instruction, 0.5 cycles/column — 2x the fp32r MAC rate).

Precision: every GEMM uses a 3-term hi/lo split, all at one shared scale so
the three products accumulate in a single PSUM chain:
    A@W ~= Ah@Wh + Al@Wh + Ah@Wl      (drops only the Al@Wl term, ~7e-4)
with Ah = e4m3(A*S), Al = e4m3(A*S - Ah). End-to-end rel err ~2e-3 vs the
2e-2 gate. x/w splits happen on host; the phase-A output s is split on
device (ACT copy for the hi part, DVE subtract for the residual).

Layouts are pre-arranged on host so every matmul operand is a direct SBUF
slice: stationary tiles [128, 2, 128] (DoubleRow K-pair x M), moving tiles
[128, 2, tw]. s_hi stays resident in SBUF; s_lo spills to DRAM and streams
back during phase B (bandwidth is far under the PE time either way).
"""

import os
import sys
from contextlib import ExitStack

import numpy as np
import ml_dtypes

for _p in ("/opt/trn_rl_repo", "/root/.axon_site/_ro/trn_rl_repo"):
    if os.path.isdir(_p) and _p not in sys.path:
        sys.path.insert(0, _p)

import concourse.mybir as mybir
import concourse.tile as tile
from concourse.tile_rust import add_dep_helper
from concourse import bacc
from concourse.bass_utils import run_bass_kernel_spmd

FP32 = mybir.dt.float32
FP8 = mybir.dt.float8e4
ALU = mybir.AluOpType
AF = mybir.ActivationFunctionType
DR = mybir.MatmulPerfMode.DoubleRow
E4NP = ml_dtypes.float8_e4m3

E = 8            # experts == cores
C = 1024         # model dim
H0 = 2728        # ffn hidden dim
NKC = C // 256   # 4 DoubleRow contraction tiles over C
KH = 22          # 128-row h tiles (padded H)
NKH = KH // 2    # 11 DoubleRow contraction tiles over padded H
HP = KH * 128    # 2816
KC8 = C // 128   # 8 output c tiles
TT = 512         # max token tile (fp32 PSUM bank = 512 floats)
CAP_MAX = 2304   # per-launch token cap (SBUF budget); split into runs beyond

# fp8 scale factors. All hi/lo parts share their tensor's scale so the three
# split products accumulate in one PSUM chain.
SX = 16.0        # x*16: |x|<5.1 -> <82, x_lo ~0.4 (normal range)
SW = 1024.0      # w*1024: |w|<0.11 -> <113
SH = 8.0         # s*8: |s|<12 -> <96 (clip-safe), s_lo ~0.07
SA = SX * SW     # phase-A psum scale
SB = SH * SW     # phase-B psum scale

_CACHE = {}
_WCACHE = {}
LAST_RESULTS = None

# startup-schedule knobs (fixed by a TimelineSim sweep)
XH_SCALAR = (1, 3)      # xh tile indices that ride the ACT queue
W0SPLIT = True          # split whi[0] into w1/w3 halves around xh0
RAMP = (128, 256, 256)  # leading token-tile widths


def _token_tiles(cap):
    # sub-512 tiles go FIRST (ascending): the first matmuls' DMA
    # dependencies are smaller, so the PE starts (and ramps) earlier. A
    # 128-wide leader is fine at fp8-DR (no narrow-tile rate penalty).
    ramp = list(RAMP)
    while sum(ramp) > max(0, cap - 256) and len(ramp) > 1:
        ramp.pop()
    widths = list(ramp)
    left = cap - sum(ramp)
    if left % TT:
        widths.append(left % TT)
    widths += [TT] * (left // TT)
    widths.sort()
    tiles = []
    off = 0
    for w in widths:
        tiles.append((off, w))
        off += w
    return tiles


def _split8(a):
    """a is pre-scaled fp32; return (hi, lo) e4m3 arrays at the same scale."""
    hi = np.clip(a, -240.0, 240.0).astype(E4NP)
    lo = (a - hi.astype(np.float32)).astype(E4NP)
    return hi, lo


def _build(cap):
    """Build + compile the SPMD program for `cap` tokens per core."""
    assert cap % 128 == 0
    tiles = _token_tiles(cap)
    last = len(tiles) - 1
    nc = bacc.Bacc("TRN2", target_bir_lowering=False, debug=False, num_devices=E)

    # x packed tile-major: per (partition, token-tile) the [NKC, 2, tw]
    # block is contiguous, so every tile's DMA moves >=2KB chunks (chunks
    # under 512B pay a 2x DMA-bus penalty in HW)
    xh = nc.dram_tensor("xh", [128, NKC * 2 * cap], FP8, kind="ExternalInput").ap()
    xl = nc.dram_tensor("xl", [128, NKC * 2 * cap], FP8, kind="ExternalInput").ap()
    # w1h+w3h (resp. w1l+w3l) fused per h-tile: one DMA instead of two
    # (fixed cost per DMA dominates these small transfers)
    whi = nc.dram_tensor("whi", [KH, 128, 2, NKC, 2, 128], FP8, kind="ExternalInput").ap()
    wlo = nc.dram_tensor("wlo", [KH, 128, 2, NKC, 2, 128], FP8, kind="ExternalInput").ap()
    w2h = nc.dram_tensor("w2h", [KC8, 128, NKH, 2, 128], FP8, kind="ExternalInput").ap()
    w2l = nc.dram_tensor("w2l", [KC8, 128, NKH, 2, 128], FP8, kind="ExternalInput").ap()
    gsc = nc.dram_tensor("gsc", [cap], FP32, kind="ExternalInput").ap()
    yt = nc.dram_tensor("yt", [KC8, 128, cap], FP32, kind="ExternalOutput").ap()

    with tile.TileContext(nc) as tc, ExitStack() as top:
        dramp = top.enter_context(tc.tile_pool(name="dram", bufs=1, space="DRAM"))
        # one scratch tensor per token tile so the phase-B reload of tile t
        # only depends on tile t's writes, not the whole phase A
        ntile = len(tiles)
        slo_dram = [dramp.tile([128, NKH, 2, TT], FP8, tag=f"slo{t}",
                               name=f"slo_dram{t}")
                    for t in range(ntile)]

        shp = top.enter_context(tc.tile_pool(name="sres", bufs=1))
        s_hi = shp.tile([128, NKH, 2, cap], FP8)

        # w2 resident for the whole kernel; loads interleaved into phase A's
        # h-loop so they hide behind compute without starving startup DMA
        w2p = top.enter_context(tc.tile_pool(name="w2res", bufs=1))
        w2h_sb = [w2p.tile([128, NKH, 2, 128], FP8, tag=f"w2h{c}",
                           name=f"w2h_sb{c}") for c in range(KC8)]
        w2l_sb = [w2p.tile([128, NKH, 2, 128], FP8, tag=f"w2l{c}",
                           name=f"w2l_sb{c}") for c in range(KC8)]
        w2_loads = [(w2h_sb[c], w2h[c]) for c in range(KC8)] + \
                   [(w2l_sb[c], w2l[c]) for c in range(KC8)]

        gbc = top.enter_context(tc.tile_pool(name="gbc", bufs=1))
        g_sb = []

        def emit_g():
            # gate-weight rows: tiny loads + partition broadcasts on the
            # SWDGE queue, emitted mid-phase-A where that queue has slack —
            # NOT at the phase boundary, where they'd sit behind the s_lo
            # write backlog and stall the first y-multiplies
            for t, (to, tw) in enumerate(tiles):
                grow = gbc.tile([1, TT], FP32, tag="grow", name=f"grow{t}",
                                bufs=2)
                nc.gpsimd.dma_start(grow[0:1, :tw], gsc[to:to + tw])
                gt = gbc.tile([128, tw], FP32, tag=f"g{t}", name=f"g_sb{t}")
                nc.gpsimd.partition_broadcast(gt[:], grow[0:1, :tw])
                g_sb.append(gt)

        # phase B's PSUM pool is allocated up front so it lands in banks
        # disjoint from phase A's — otherwise B's first chain waits ~1us
        # for A's tail to release a recycled bank
        psb = top.enter_context(tc.tile_pool(name="psB", bufs=3, space="PSUM"))
        anchor = None
        with ExitStack() as pha:
            xp = pha.enter_context(tc.tile_pool(name="xres", bufs=1))
            xh_sb = [xp.tile([128, NKC, 2, tw], FP8, tag=f"xh{t}",
                             name=f"xh_sb{t}") for t, (to, tw) in enumerate(tiles)]
            xl_sb = [xp.tile([128, NKC, 2, tw], FP8, tag=f"xl{t}",
                             name=f"xl_sb{t}") for t, (to, tw) in enumerate(tiles)]
            wst = pha.enter_context(tc.tile_pool(name="wst", bufs=4))

            def walloc(h):
                return (
                    wst.tile([128, 2, NKC, 2, 128], FP8, tag="whi", name=f"whi_{h}"),
                    wst.tile([128, 2, NKC, 2, 128], FP8, tag="wlo", name=f"wlo_{h}"),
                )

            # startup streams split across the two free queues in exact
            # consumption order (per-queue DMA processing is FIFO): SYNC
            # carries the hi parts (consumed first in every chain) + xh;
            # the gpsimd/SWDGE queue carries xl + the fused lo parts. The
            # ACT queue must stay empty here: each DMA on it would occupy
            # the ACT sequencer ~1.3us and push the silu/quantize chain
            # (and with it PSUM recycling) out by that much. x rides ahead
            # of the h>=1 weights: each xh tile is consumed ~1us after the
            # previous, while w[h] only gates the next 11us-long h-sweep.
            def wload_hi(h, wt):
                nc.sync.dma_start(wt[0][:], whi[h])

            def wload_lo(h, wt):
                nc.gpsimd.dma_start(wt[1][:], wlo[h])

            # only h0/h1 preload: the startup DMA wall is the global
            # DMA-engine bandwidth, so deferring h2+ weights (needed only
            # ~22us in) out of the startup window shrinks the stall
            npre = min(2, KH)
            w_cur = {h: walloc(h) for h in range(npre)}
            # h0's hi weights optionally split in two: the first chain only
            # needs the w1 half, so it rides ahead of xh0, w3 follows
            if W0SPLIT:
                nc.sync.dma_start(w_cur[0][0][:, 0], whi[0][:, 0])
            else:
                wload_hi(0, w_cur[0])
            for ti, (to, tw) in enumerate(tiles):
                # some xh tiles ride the ACT queue: a third startup channel
                # (ACT has no compute until the first silu lands, well
                # after these triggers retire)
                xq = nc.scalar if ti in XH_SCALAR else nc.sync
                xq.dma_start(
                    xh_sb[ti][:],
                    xh[:, 8 * to:8 * (to + tw)].rearrange(
                        "p (k i t) -> p k i t", k=NKC, i=2))
                nc.gpsimd.dma_start(
                    xl_sb[ti][:],
                    xl[:, 8 * to:8 * (to + tw)].rearrange(
                        "p (k i t) -> p k i t", k=NKC, i=2))
                if ti == 0:
                    if W0SPLIT:
                        nc.sync.dma_start(w_cur[0][0][:, 1], whi[0][:, 1])
                    wload_lo(0, w_cur[0])
                    wload_hi(1, w_cur[1])
                    wload_lo(1, w_cur[1])

            psa = pha.enter_context(tc.tile_pool(name="psA", bufs=2, space="PSUM"))
            stg = pha.enter_context(tc.tile_pool(name="stg", bufs=3))
            # deep staging: s_lo DMA-out rides the busy SWDGE queue, so the
            # writes may lag the compute by several (h,t) groups
            slop = pha.enter_context(tc.tile_pool(name="slo", bufs=12))

            def emit_ht(h, t, wt):
                nonlocal anchor
                whi_t, wlo_t = wt
                to, tw = tiles[t]
                hd, blk = divmod(h, 2)
                p1 = psa.tile([128, TT], FP32, tag="p1", name=f"p1_{h}_{t}")
                p3 = psa.tile([128, TT], FP32, tag="p3", name=f"p3_{h}_{t}")
                xh_t, xl_t = xh_sb[t], xl_sb[t]
                for pp, wh_, wl_ in ((p1, whi_t[:, 0], wlo_t[:, 0]),
                                     (p3, whi_t[:, 1], wlo_t[:, 1])):
                    n = 0
                    for xs_, ws_ in ((xh_t, wh_), (xl_t, wh_), (xh_t, wl_)):
                        for kd in range(NKC):
                            anchor = nc.tensor.matmul(
                                pp[:, :tw], ws_[:, kd], xs_[:, kd],
                                start=(n == 0), stop=(n == 3 * NKC - 1),
                                perf_mode=DR)
                            n += 1
                sa = stg.tile([128, TT], FP32, tag="sa", name=f"sa{h}_{t}")
                nc.scalar.activation(sa[:, :tw], p1[:, :tw], AF.Silu,
                                     scale=1.0 / SA)
                t1 = stg.tile([128, TT], FP32, tag="t1", name=f"t1_{h}_{t}")
                acc = stg.tile([128, 1], FP32, tag="acc", name=f"acc{h}_{t}")
                nc.vector.affine_mul_reduce(t1[:, :tw], acc[:], p3[:, :tw],
                                            sa[:, :tw], SH / SA, 0.0)
                hi_sl = s_hi[:, hd, blk, to:to + tw]
                nc.scalar.activation(hi_sl, t1[:, :tw], AF.Copy)
                slo = slop.tile([128, TT], FP8, tag="slo", name=f"slo{h}_{t}")
                nc.vector.tensor_tensor(slo[:, :tw], t1[:, :tw], hi_sl,
                                        op=ALU.subtract)
                nc.gpsimd.dma_start(slo_dram[t][:, hd, blk, :tw],
                                    slo[:, :tw])

            # emission: h0/h1 interleaved token-major — each arriving x tile
            # feeds 2x the PE work, halving the startup feed-rate demand on
            # the global DMA engines
            for t in range(len(tiles)):
                for h in range(npre):
                    emit_ht(h, t, w_cur[h])
            for h in range(npre, KH):
                wt = walloc(h)
                wload_hi(h, wt)
                wload_lo(h, wt)
                w_cur[h] = wt
                j = h - npre
                if j < len(w2_loads):
                    dst, src = w2_loads[j]
                    w2dma = nc.gpsimd.dma_start(dst[:], src)
                    add_dep_helper(w2dma.ins, anchor.ins,
                                   reason="delay w2 prefetch")
                if h == 6:
                    emit_g()
                for t in range(len(tiles)):
                    emit_ht(h, t, w_cur[h])
            # two stragglers (KH - npre = 18 slots for 16 w2 loads) — none

        # ---- phase B: y = 3-term(s @ w2.T) * g ----
        with ExitStack() as phb:
            sin = phb.enter_context(tc.tile_pool(name="sin", bufs=2))
            yp = phb.enter_context(tc.tile_pool(name="yst", bufs=4))
            # big tiles in the middle; the smallest tile LAST so the final
            # y writeback (which trails the last matmul) is the shortest
            b_order = sorted(range(len(tiles)),
                             key=lambda i: (-tiles[i][1], i))
            b_order = b_order[:-1] + [b_order[-1]]
            sm = min(range(len(tiles)), key=lambda i: tiles[i][1])
            b_order = [i for i in b_order if i != sm] + [sm]
            # the first tile's s_lo reload rides SYNC (idle through phase A,
            # and the wait on that tile's writes resolves ~10us before the
            # A/B boundary); later tiles go per-tile on the ACT queue, which
            # frees up right at the boundary
            for bi, t in enumerate(b_order):
                to, tw = tiles[t]
                sl_t = sin.tile([128, NKH, 2, TT], FP8, tag="sl",
                                name=f"sl_sb{t}")
                q = nc.sync if bi == 0 else nc.scalar
                q.dma_start(sl_t[:, :, :, :tw], slo_dram[t][:, :, :, :tw])
                for c in range(KC8):
                    py = psb.tile([128, TT], FP32, tag="py", name=f"py{t}_{c}")
                    n = 0
                    # the s_lo group goes LAST so the chain can start before
                    # the reload DMA of this tile's s_lo has landed
                    for s_, w_ in ((s_hi, w2h_sb[c]), (s_hi, w2l_sb[c]),
                                   (None, w2h_sb[c])):
                        for hd in range(NKH):
                            mv = (sl_t[:, hd, :, :tw] if s_ is None
                                  else s_[:, hd, :, to:to + tw])
                            nc.tensor.matmul(py[:, :tw], w_[:, hd], mv,
                                             start=(n == 0),
                                             stop=(n == 3 * NKH - 1),
                                             perf_mode=DR)
                            n += 1
                    yb = yp.tile([128, TT], FP32, tag="y", name=f"yb{t}_{c}")
                    nc.vector.tensor_tensor(yb[:, :tw], py[:, :tw], g_sb[t][:],
                                            op=ALU.mult)
                    # alternate writeback queues: halves the per-queue y
                    # rate so the final transfer doesn't trail the compute.
                    # Odd c on SYNC so the last chain's y (c=7) takes the
                    # SP path, whose DGE delay is 134ns shorter than ACT's
                    yq = nc.sync if c % 2 == 1 else nc.scalar
                    yq.dma_start(yt[c, :, to:to + tw], yb[:, :tw])

    nc.compile()
    return nc


def _prep_weights(gw, w1, w2, w3):
    """Quantize + arrange all per-expert weight tensors (host, cached)."""
    wmaps = []
    for e in range(E):
        m = {}
        his, los = {}, {}
        for nm, w in (("w1", w1[e]), ("w3", w3[e])):
            wt = np.zeros((HP, C), np.float32)
            wt[:H0] = w
            hi, lo = _split8(wt * SW)
            # [HP, C] -> [KH, 128m, NKC, 2, 128p] -> [KH, 128p, NKC, 2, 128m]
            his[nm] = hi.reshape(KH, 128, NKC, 2, 128).transpose(0, 4, 2, 3, 1)
            los[nm] = lo.reshape(KH, 128, NKC, 2, 128).transpose(0, 4, 2, 3, 1)
        m["whi"] = np.ascontiguousarray(
            np.stack([his["w1"], his["w3"]], axis=2))
        m["wlo"] = np.ascontiguousarray(
            np.stack([los["w1"], los["w3"]], axis=2))
        wt = np.zeros((C, HP), np.float32)
        wt[:, :H0] = w2[e]
        hi, lo = _split8(wt * SW)
        # [C, HP] -> [KC8, 128m, NKH, 2, 128p] -> [KC8, 128p, NKH, 2, 128m]
        for part, arr in (("h", hi), ("l", lo)):
            a = arr.reshape(KC8, 128, NKH, 2, 128).transpose(0, 4, 2, 3, 1)
            m["w2" + part] = np.ascontiguousarray(a)
        wmaps.append(m)
    return wmaps


def kernel(x, gate_w, w1, w2, w3, top_k):
    global LAST_RESULTS
    x = np.asarray(x, dtype=np.float32)
    gw = np.asarray(gate_w, dtype=np.float32)
    w1 = np.asarray(w1, dtype=np.float32)
    w2 = np.asarray(w2, dtype=np.float32)
    w3 = np.asarray(w3, dtype=np.float32)
    assert int(np.asarray(top_k)) == 2
    Bb, T, Cc = x.shape
    N = Bb * T
    assert Cc == C and w1.shape == (E, H0, C)

    xf = np.ascontiguousarray(x.reshape(N, C))
    # Router on host (exact fp32): top-2 selection + softmax combine weights.
    logits = xf @ gw.T
    order = np.argsort(-logits, axis=1, kind="stable")[:, :2]
    vals = np.take_along_axis(logits, order, axis=1)
    sw = np.exp(vals - vals.max(axis=1, keepdims=True))
    sw /= sw.sum(axis=1, keepdims=True)
    tok, gtok = [], []
    for e in range(E):
        sel = order == e
        idx = np.nonzero(sel.any(axis=1))[0]
        tok.append(idx)
        gtok.append(sw[sel].astype(np.float32))

    key = (w1.shape, float(w1[0, 0, :8].sum()), float(w2[-1, -1, :8].sum()),
           float(w3[0, -1, :8].sum()))
    wm = _WCACHE.get(key)
    if wm is None:
        wm = _prep_weights(gw, w1, w2, w3)
        _WCACHE.clear()
        _WCACHE[key] = wm

    # quantize x once (full token set), dispatch indexes the fp8 arrays
    xs = xf * SX
    xh_full, xl_full = _split8(xs)

    out = np.zeros((N, C), np.float32)
    nchunk = (max(t.size for t in tok) + CAP_MAX - 1) // CAP_MAX
    for ci in range(nchunk):
        tokc, gc = [], []
        for e in range(E):
            lo_ = (ci * tok[e].size) // nchunk
            hi_ = ((ci + 1) * tok[e].size) // nchunk
            tokc.append(tok[e][lo_:hi_])
            gc.append(gtok[e][lo_:hi_])
        cap = max(TT, ((max(t.size for t in tokc) + 127) // 128) * 128)
        if cap not in _CACHE:
            _CACHE[cap] = _build(cap)
        nc = _CACHE[cap]
        in_maps = []
        for e in range(E):
            idx = tokc[e]
            n = idx.size
            im = dict(wm[e])
            tls = _token_tiles(cap)
            for nm, full in (("xh", xh_full), ("xl", xl_full)):
                xe = np.zeros((cap, C), E4NP)
                xe[:n] = full[idx]
                # [cap, C] -> [128p, NKC, 2, cap] with c = kd*256+i*128+p,
                # then packed tile-major: per partition the [NKC, 2, tw]
                # block of each token tile is contiguous
                a = xe.T.reshape(NKC, 2, 128, cap).transpose(2, 0, 1, 3)
                im[nm] = np.concatenate(
                    [np.ascontiguousarray(a[:, :, :, to:to + tw]).reshape(128, -1)
                     for to, tw in tls], axis=1)
            g = np.zeros(cap, np.float32)
            g[:n] = gc[e] / SB
            im["gsc"] = g
            in_maps.append(im)

        res = run_bass_kernel_spmd(nc, in_maps, core_ids=list(range(E)))
        LAST_RESULTS = res

        for e in range(E):
            idx = tokc[e]
            n = idx.size
            ye = res.results[e]["yt"].reshape(C, cap).T
            out[idx] += ye[:n]
    return out.reshape(Bb, T, C)


# revision 66
# speedup vs baseline: 1.0230x; 1.0226x over previous
"""Expert-parallel MoE layer for Trainium2 (Bass/Tile, 8 NeuronCores).

Strategy (hardcoded for B=4, T=2048, C=1024, E=8, H=2728, top_k=2):
  - Expert-parallel: core e owns expert e's weights (w1/w2/w3[e]).
  - Host computes the router (top-2 ids AND the softmax combine weights in
    exact fp32) and performs the all-to-all token dispatch/combine as the
    shard/unshard step. The per-token gate weight rides in as a small fp32
    vector, pre-scaled by the fp8 scale factors.
  - Each core computes the full expert FFN y = (silu(x@w1.T) * (x@w3.T))
    @ w2.T * g for its tokens, entirely in fp8-e4m3 DoubleRow matmuls
    (K=256 per instruction, 0.5 cycles/column — 2x the fp32r MAC rate).

Precision: every GEMM uses a 3-term hi/lo split, all at one shared scale so
the three products accumulate in a single PSUM chain:
    A@W ~= Ah@Wh + Al@Wh + Ah@Wl      (drops only the Al@Wl term, ~7e-4)
with Ah = e4m3(A*S), Al = e4m3(A*S - Ah). End-to-end rel err ~2e-3 vs the
2e-2 gate. x/w splits happen on host; the phase-A output s is split on
device (ACT copy for the hi part, DVE subtract for the residual).

Layouts are pre-arranged on host so every matmul operand is a direct SBUF
slice: stationary tiles [128, 2, 128] (DoubleRow K-pair x M), moving tiles
[128, 2, tw]. s_hi stays resident in SBUF; s_lo spills to DRAM and streams
back during phase B (bandwidth is far under the PE time either way).
"""

import os
import sys
from contextlib import ExitStack

import numpy as np
import ml_dtypes

for _p in ("/opt/trn_rl_repo", "/root/.axon_site/_ro/trn_rl_repo"):
    if os.path.isdir(_p) and _p not in sys.path:
        sys.path.insert(0, _p)

import concourse.mybir as mybir
import concourse.tile as tile
from concourse.tile_rust import add_dep_helper
from concourse import bacc
from concourse.bass_utils import run_bass_kernel_spmd

FP32 = mybir.dt.float32
FP8 = mybir.dt.float8e4
ALU = mybir.AluOpType
AF = mybir.ActivationFunctionType
DR = mybir.MatmulPerfMode.DoubleRow
E4NP = ml_dtypes.float8_e4m3

E = 8            # experts == cores
C = 1024         # model dim
H0 = 2728        # ffn hidden dim
NKC = C // 256   # 4 DoubleRow contraction tiles over C
KH = 22          # 128-row h tiles (padded H)
NKH = KH // 2    # 11 DoubleRow contraction tiles over padded H
HP = KH * 128    # 2816
KC8 = C // 128   # 8 output c tiles
TT = 512         # max token tile (fp32 PSUM bank = 512 floats)
CAP_MAX = 2304   # per-launch token cap (SBUF budget); split into runs beyond

# fp8 scale factors. All hi/lo parts share their tensor's scale so the three
# split products accumulate in one PSUM chain.
SX = 16.0        # x*16: |x|<5.1 -> <82, x_lo ~0.4 (normal range)
SW = 1024.0      # w*1024: |w|<0.11 -> <113
SH = 8.0         # s*8: |s|<12 -> <96 (clip-safe), s_lo ~0.07
SA = SX * SW     # phase-A psum scale
SB = SH * SW     # phase-B psum scale

_CACHE = {}
_WCACHE = {}
LAST_RESULTS = None

# startup-schedule knobs (fixed by a TimelineSim sweep)
XH_SCALAR = (1, 3)      # xh tile indices that ride the ACT queue
W0SPLIT = True          # split whi[0] into w1/w3 halves around xh0
RAMP = (128, 256, 256)  # leading token-tile widths


def _token_tiles(cap):
    # sub-512 tiles go FIRST (ascending): the first matmuls' DMA
    # dependencies are smaller, so the PE starts (and ramps) earlier. A
    # 128-wide leader is fine at fp8-DR (no narrow-tile rate penalty).
    ramp = list(RAMP)
    while sum(ramp) > max(0, cap - 256) and len(ramp) > 1:
        ramp.pop()
    widths = list(ramp)
    left = cap - sum(ramp)
    if left % TT:
        widths.append(left % TT)
    widths += [TT] * (left // TT)
    widths.sort()
    tiles = []
    off = 0
    for w in widths:
        tiles.append((off, w))
        off += w
    return tiles


def _split8(a):
    """a is pre-scaled fp32; return (hi, lo) e4m3 arrays at the same scale."""
    hi = np.clip(a, -240.0, 240.0).astype(E4NP)
    lo = (a - hi.astype(np.float32)).astype(E4NP)
    return hi, lo


def _build(cap):
    """Build + compile the SPMD program for `cap` tokens per core."""
    assert cap % 128 == 0
    tiles = _token_tiles(cap)
    last = len(tiles) - 1
    nc = bacc.Bacc("TRN2", target_bir_lowering=False, debug=False, num_devices=E)

    # x packed tile-major: per (partition, token-tile) the [NKC, 2, tw]
    # block is contiguous, so every tile's DMA moves >=2KB chunks (chunks
    # under 512B pay a 2x DMA-bus penalty in HW)
    xh = nc.dram_tensor("xh", [128, NKC * 2 * cap], FP8, kind="ExternalInput").ap()
    xl = nc.dram_tensor("xl", [128, NKC * 2 * cap], FP8, kind="ExternalInput").ap()
    # w1h+w3h (resp. w1l+w3l) fused per h-tile: one DMA instead of two
    # (fixed cost per DMA dominates these small transfers)
    whi = nc.dram_tensor("whi", [KH, 128, 2, NKC, 2, 128], FP8, kind="ExternalInput").ap()
    wlo = nc.dram_tensor("wlo", [KH, 128, 2, NKC, 2, 128], FP8, kind="ExternalInput").ap()
    w2h = nc.dram_tensor("w2h", [KC8, 128, NKH, 2, 128], FP8, kind="ExternalInput").ap()
    w2l = nc.dram_tensor("w2l", [KC8, 128, NKH, 2, 128], FP8, kind="ExternalInput").ap()
    gsc = nc.dram_tensor("gsc", [cap], FP32, kind="ExternalInput").ap()
    yt = nc.dram_tensor("yt", [KC8, 128, cap], FP32, kind="ExternalOutput").ap()

    with tile.TileContext(nc) as tc, ExitStack() as top:
        dramp = top.enter_context(tc.tile_pool(name="dram", bufs=1, space="DRAM"))
        # one scratch tensor per token tile so the phase-B reload of tile t
        # only depends on tile t's writes, not the whole phase A
        ntile = len(tiles)
        slo_dram = [dramp.tile([128, NKH, 2, TT], FP8, tag=f"slo{t}",
                               name=f"slo_dram{t}")
                    for t in range(ntile)]

        shp = top.enter_context(tc.tile_pool(name="sres", bufs=1))
        s_hi = shp.tile([128, NKH, 2, cap], FP8)

        # w2 resident for the whole kernel; loads interleaved into phase A's
        # h-loop so they hide behind compute without starving startup DMA
        w2p = top.enter_context(tc.tile_pool(name="w2res", bufs=1))
        w2h_sb = [w2p.tile([128, NKH, 2, 128], FP8, tag=f"w2h{c}",
                           name=f"w2h_sb{c}") for c in range(KC8)]
        w2l_sb = [w2p.tile([128, NKH, 2, 128], FP8, tag=f"w2l{c}",
                           name=f"w2l_sb{c}") for c in range(KC8)]
        w2_loads = [(w2h_sb[c], w2h[c]) for c in range(KC8)] + \
                   [(w2l_sb[c], w2l[c]) for c in range(KC8)]

        gbc = top.enter_context(tc.tile_pool(name="gbc", bufs=1))
        g_sb = []

        def emit_g():
            # gate-weight rows: tiny loads + partition broadcasts on the
            # SWDGE queue, emitted mid-phase-A where that queue has slack —
            # NOT at the phase boundary, where they'd sit behind the s_lo
            # write backlog and stall the first y-multiplies
            for t, (to, tw) in enumerate(tiles):
                grow = gbc.tile([1, TT], FP32, tag="grow", name=f"grow{t}",
                                bufs=2)
                nc.gpsimd.dma_start(grow[0:1, :tw], gsc[to:to + tw])
                gt = gbc.tile([128, tw], FP32, tag=f"g{t}", name=f"g_sb{t}")
                nc.gpsimd.partition_broadcast(gt[:], grow[0:1, :tw])
                g_sb.append(gt)

        # phase B's PSUM pool is allocated up front so it lands in banks
        # disjoint from phase A's — otherwise B's first chain waits ~1us
        # for A's tail to release a recycled bank
        psb = top.enter_context(tc.tile_pool(name="psB", bufs=3, space="PSUM"))
        anchor = None
        with ExitStack() as pha:
            xp = pha.enter_context(tc.tile_pool(name="xres", bufs=1))
            xh_sb = [xp.tile([128, NKC, 2, tw], FP8, tag=f"xh{t}",
                             name=f"xh_sb{t}") for t, (to, tw) in enumerate(tiles)]
            xl_sb = [xp.tile([128, NKC, 2, tw], FP8, tag=f"xl{t}",
                             name=f"xl_sb{t}") for t, (to, tw) in enumerate(tiles)]
            wst = pha.enter_context(tc.tile_pool(name="wst", bufs=4))

            def walloc(h):
                return (
                    wst.tile([128, 2, NKC, 2, 128], FP8, tag="whi", name=f"whi_{h}"),
                    wst.tile([128, 2, NKC, 2, 128], FP8, tag="wlo", name=f"wlo_{h}"),
                )

            # startup streams split across the two free queues in exact
            # consumption order (per-queue DMA processing is FIFO): SYNC
            # carries the hi parts (consumed first in every chain) + xh;
            # the gpsimd/SWDGE queue carries xl + the fused lo parts. The
            # ACT queue must stay empty here: each DMA on it would occupy
            # the ACT sequencer ~1.3us and push the silu/quantize chain
            # (and with it PSUM recycling) out by that much. x rides ahead
            # of the h>=1 weights: each xh tile is consumed ~1us after the
            # previous, while w[h] only gates the next 11us-long h-sweep.
            def wload_hi(h, wt):
                nc.sync.dma_start(wt[0][:], whi[h])

            def wload_lo(h, wt):
                nc.gpsimd.dma_start(wt[1][:], wlo[h])

            # only h0/h1 preload: the startup DMA wall is the global
            # DMA-engine bandwidth, so deferring h2+ weights (needed only
            # ~22us in) out of the startup window shrinks the stall
            npre = min(2, KH)
            w_cur = {h: walloc(h) for h in range(npre)}
            # h0's hi weights optionally split in two: the first chain only
            # needs the w1 half, so it rides ahead of xh0, w3 follows
            if W0SPLIT:
                nc.sync.dma_start(w_cur[0][0][:, 0], whi[0][:, 0])
            else:
                wload_hi(0, w_cur[0])
            for ti, (to, tw) in enumerate(tiles):
                # some xh tiles ride the ACT queue: a third startup channel
                # (ACT has no compute until the first silu lands, well
                # after these triggers retire)
                xq = nc.scalar if ti in XH_SCALAR else nc.sync
                xq.dma_start(
                    xh_sb[ti][:],
                    xh[:, 8 * to:8 * (to + tw)].rearrange(
                        "p (k i t) -> p k i t", k=NKC, i=2))
                nc.gpsimd.dma_start(
                    xl_sb[ti][:],
                    xl[:, 8 * to:8 * (to + tw)].rearrange(
                        "p (k i t) -> p k i t", k=NKC, i=2))
                if ti == 0:
                    if W0SPLIT:
                        nc.sync.dma_start(w_cur[0][0][:, 1], whi[0][:, 1])
                    wload_lo(0, w_cur[0])
                    wload_hi(1, w_cur[1])
                    wload_lo(1, w_cur[1])

            psa = pha.enter_context(tc.tile_pool(name="psA", bufs=2, space="PSUM"))
            stg = pha.enter_context(tc.tile_pool(name="stg", bufs=3))
            # deep staging: s_lo DMA-out rides the busy SWDGE queue, so the
            # writes may lag the compute by several (h,t) groups
            slop = pha.enter_context(tc.tile_pool(name="slo", bufs=12))

            def emit_ht(h, t, wt):
                nonlocal anchor
                whi_t, wlo_t = wt
                to, tw = tiles[t]
                hd, blk = divmod(h, 2)
                p1 = psa.tile([128, TT], FP32, tag="p1", name=f"p1_{h}_{t}")
                p3 = psa.tile([128, TT], FP32, tag="p3", name=f"p3_{h}_{t}")
                xh_t, xl_t = xh_sb[t], xl_sb[t]
                # the last two h-tiles (168 real rows of 2728) drop the
                # x-residual correction term: costs ~6.4e-3 of the 2e-2
                # error budget (error scales with sqrt of the affected
                # h-fraction) and deletes 8 cyc/token of PE floor
                terms = ((xh_t, 0), (xl_t, 0), (xh_t, 1))
                if h >= KH - 2:
                    terms = ((xh_t, 0), (xh_t, 1))
                nmm = len(terms) * NKC
                for pp, wpair in ((p1, (whi_t[:, 0], wlo_t[:, 0])),
                                  (p3, (whi_t[:, 1], wlo_t[:, 1]))):
                    n = 0
                    for xs_, wi in terms:
                        ws_ = wpair[wi]
                        for kd in range(NKC):
                            anchor = nc.tensor.matmul(
                                pp[:, :tw], ws_[:, kd], xs_[:, kd],
                                start=(n == 0), stop=(n == nmm - 1),
                                perf_mode=DR)
                            n += 1
                sa = stg.tile([128, TT], FP32, tag="sa", name=f"sa{h}_{t}")
                nc.scalar.activation(sa[:, :tw], p1[:, :tw], AF.Silu,
                                     scale=1.0 / SA)
                t1 = stg.tile([128, TT], FP32, tag="t1", name=f"t1_{h}_{t}")
                acc = stg.tile([128, 1], FP32, tag="acc", name=f"acc{h}_{t}")
                nc.vector.affine_mul_reduce(t1[:, :tw], acc[:], p3[:, :tw],
                                            sa[:, :tw], SH / SA, 0.0)
                hi_sl = s_hi[:, hd, blk, to:to + tw]
                nc.scalar.activation(hi_sl, t1[:, :tw], AF.Copy)
                slo = slop.tile([128, TT], FP8, tag="slo", name=f"slo{h}_{t}")
                nc.vector.tensor_tensor(slo[:, :tw], t1[:, :tw], hi_sl,
                                        op=ALU.subtract)
                nc.gpsimd.dma_start(slo_dram[t][:, hd, blk, :tw],
                                    slo[:, :tw])

            # emission: h0/h1 interleaved token-major — each arriving x tile
            # feeds 2x the PE work, halving the startup feed-rate demand on
            # the global DMA engines
            for t in range(len(tiles)):
                for h in range(npre):
                    emit_ht(h, t, w_cur[h])
            for h in range(npre, KH):
                wt = walloc(h)
                wload_hi(h, wt)
                wload_lo(h, wt)
                w_cur[h] = wt
                j = h - npre
                if j < len(w2_loads):
                    dst, src = w2_loads[j]
                    w2dma = nc.gpsimd.dma_start(dst[:], src)
                    add_dep_helper(w2dma.ins, anchor.ins,
                                   reason="delay w2 prefetch")
                if h == 6:
                    emit_g()
                for t in range(len(tiles)):
                    emit_ht(h, t, w_cur[h])
            # two stragglers (KH - npre = 18 slots for 16 w2 loads) — none

        # ---- phase B: y = 3-term(s @ w2.T) * g ----
        with ExitStack() as phb:
            sin = phb.enter_context(tc.tile_pool(name="sin", bufs=2))
            yp = phb.enter_context(tc.tile_pool(name="yst", bufs=4))
            # big tiles in the middle; the smallest tile LAST so the final
            # y writeback (which trails the last matmul) is the shortest
            b_order = sorted(range(len(tiles)),
                             key=lambda i: (-tiles[i][1], i))
            b_order = b_order[:-1] + [b_order[-1]]
            sm = min(range(len(tiles)), key=lambda i: tiles[i][1])
            b_order = [i for i in b_order if i != sm] + [sm]
            # the first tile's s_lo reload rides SYNC (idle through phase A,
            # and the wait on that tile's writes resolves ~10us before the
            # A/B boundary); later tiles go per-tile on the ACT queue, which
            # frees up right at the boundary
            for bi, t in enumerate(b_order):
                to, tw = tiles[t]
                sl_t = sin.tile([128, NKH, 2, TT], FP8, tag="sl",
                                name=f"sl_sb{t}")
                q = nc.sync if bi == 0 else nc.scalar
                q.dma_start(sl_t[:, :, :, :tw], slo_dram[t][:, :, :, :tw])
                for c in range(KC8):
                    py = psb.tile([128, TT], FP32, tag="py", name=f"py{t}_{c}")
                    n = 0
                    # the s_lo group goes LAST so the chain can start before
                    # the reload DMA of this tile's s_lo has landed. The w2
                    # residual term skips the pad-heavy last hd tile (168
                    # real rows): ~6.5e-3 error for 4 cyc/token of floor
                    for s_, w_, nhd in ((s_hi, w2h_sb[c], NKH),
                                        (s_hi, w2l_sb[c], NKH - 1),
                                        (None, w2h_sb[c], NKH)):
                        for hd in range(nhd):
                            mv = (sl_t[:, hd, :, :tw] if s_ is None
                                  else s_[:, hd, :, to:to + tw])
                            nc.tensor.matmul(py[:, :tw], w_[:, hd], mv,
                                             start=(n == 0),
                                             stop=(n == 3 * NKH - 2),
                                             perf_mode=DR)
                            n += 1
                    yb = yp.tile([128, TT], FP32, tag="y", name=f"yb{t}_{c}")
                    nc.vector.tensor_tensor(yb[:, :tw], py[:, :tw], g_sb[t][:],
                                            op=ALU.mult)
                    # alternate writeback queues: halves the per-queue y
                    # rate so the final transfer doesn't trail the compute.
                    # Odd c on SYNC so the last chain's y (c=7) takes the
                    # SP path, whose DGE delay is 134ns shorter than ACT's
                    yq = nc.sync if c % 2 == 1 else nc.scalar
                    yq.dma_start(yt[c, :, to:to + tw], yb[:, :tw])

    nc.compile()
    return nc


def _prep_weights(gw, w1, w2, w3):
    """Quantize + arrange all per-expert weight tensors (host, cached)."""
    wmaps = []
    for e in range(E):
        m = {}
        his, los = {}, {}
        for nm, w in (("w1", w1[e]), ("w3", w3[e])):
            wt = np.zeros((HP, C), np.float32)
            wt[:H0] = w
            hi, lo = _split8(wt * SW)
            # [HP, C] -> [KH, 128m, NKC, 2, 128p] -> [KH, 128p, NKC, 2, 128m]
            his[nm] = hi.reshape(KH, 128, NKC, 2, 128).transpose(0, 4, 2, 3, 1)
            los[nm] = lo.reshape(KH, 128, NKC, 2, 128).transpose(0, 4, 2, 3, 1)
        m["whi"] = np.ascontiguousarray(
            np.stack([his["w1"], his["w3"]], axis=2))
        m["wlo"] = np.ascontiguousarray(
            np.stack([los["w1"], los["w3"]], axis=2))
        wt = np.zeros((C, HP), np.float32)
        wt[:, :H0] = w2[e]
        hi, lo = _split8(wt * SW)
        # [C, HP] -> [KC8, 128m, NKH, 2, 128p] -> [KC8, 128p, NKH, 2, 128m]
        for part, arr in (("h", hi), ("l", lo)):
            a = arr.reshape(KC8, 128, NKH, 2, 128).transpose(0, 4, 2, 3, 1)
            m["w2" + part] = np.ascontiguousarray(a)
        wmaps.append(m)
    return wmaps


def kernel(x, gate_w, w1, w2, w3, top_k):
    global LAST_RESULTS
    x = np.asarray(x, dtype=np.float32)
    gw = np.asarray(gate_w, dtype=np.float32)
    w1 = np.asarray(w1, dtype=np.float32)
    w2 = np.asarray(w2, dtype=np.float32)
    w3 = np.asarray(w3, dtype=np.float32)
    assert int(np.asarray(top_k)) == 2
    Bb, T, Cc = x.shape
    N = Bb * T
    assert Cc == C and w1.shape == (E, H0, C)

    xf = np.ascontiguousarray(x.reshape(N, C))
    # Router on host (exact fp32): top-2 selection + softmax combine weights.
    logits = xf @ gw.T
    order = np.argsort(-logits, axis=1, kind="stable")[:, :2]
    vals = np.take_along_axis(logits, order, axis=1)
    sw = np.exp(vals - vals.max(axis=1, keepdims=True))
    sw /= sw.sum(axis=1, keepdims=True)
    tok, gtok = [], []
    for e in range(E):
        sel = order == e
        idx = np.nonzero(sel.any(axis=1))[0]
        tok.append(idx)
        gtok.append(sw[sel].astype(np.float32))

    key = (w1.shape, float(w1[0, 0, :8].sum()), float(w2[-1, -1, :8].sum()),
           float(w3[0, -1, :8].sum()))
    wm = _WCACHE.get(key)
    if wm is None:
        wm = _prep_weights(gw, w1, w2, w3)
        _WCACHE.clear()
        _WCACHE[key] = wm

    # quantize x once (full token set), dispatch indexes the fp8 arrays
    xs = xf * SX
    xh_full, xl_full = _split8(xs)

    out = np.zeros((N, C), np.float32)
    nchunk = (max(t.size for t in tok) + CAP_MAX - 1) // CAP_MAX
    for ci in range(nchunk):
        tokc, gc = [], []
        for e in range(E):
            lo_ = (ci * tok[e].size) // nchunk
            hi_ = ((ci + 1) * tok[e].size) // nchunk
            tokc.append(tok[e][lo_:hi_])
            gc.append(gtok[e][lo_:hi_])
        cap = max(TT, ((max(t.size for t in tokc) + 127) // 128) * 128)
        if cap not in _CACHE:
            _CACHE[cap] = _build(cap)
        nc = _CACHE[cap]
        in_maps = []
        for e in range(E):
            idx = tokc[e]
            n = idx.size
            im = dict(wm[e])
            tls = _token_tiles(cap)
            for nm, full in (("xh", xh_full), ("xl", xl_full)):
                xe = np.zeros((cap, C), E4NP)
                xe[:n] = full[idx]
                # [cap, C] -> [128p, NKC, 2, cap] with c = kd*256+i*128+p,
                # then packed tile-major: per partition the [NKC, 2, tw]
                # block of each token tile is contiguous
                a = xe.T.reshape(NKC, 2, 128, cap).transpose(2, 0, 1, 3)
                im[nm] = np.concatenate(
                    [np.ascontiguousarray(a[:, :, :, to:to + tw]).reshape(128, -1)
                     for to, tw in tls], axis=1)
            g = np.zeros(cap, np.float32)
            g[:n] = gc[e] / SB
            im["gsc"] = g
            in_maps.append(im)

        res = run_bass_kernel_spmd(nc, in_maps, core_ids=list(range(E)))
        LAST_RESULTS = res

        for e in range(E):
            idx = tokc[e]
            n = idx.size
            ye = res.results[e]["yt"].reshape(C, cap).T
            out[idx] += ye[:n]
    return out.reshape(Bb, T, C)


# revision 68
# speedup vs baseline: 1.0549x; 1.0312x over previous
"""Expert-parallel MoE layer for Trainium2 (Bass/Tile, 8 NeuronCores).

Strategy (hardcoded for B=4, T=2048, C=1024, E=8, H=2728, top_k=2):
  - Expert-parallel: core e owns expert e's weights (w1/w2/w3[e]).
  - Host computes the router (top-2 ids AND the softmax combine weights in
    exact fp32) and performs the all-to-all token dispatch/combine as the
    shard/unshard step. The per-token gate weight rides in as a small fp32
    vector, pre-scaled by the fp8 scale factors.
  - Each core computes the full expert FFN y = (silu(x@w1.T) * (x@w3.T))
    @ w2.T * g for its tokens, entirely in fp8-e4m3 DoubleRow matmuls
    (K=256 per instruction, 0.5 cycles/column — 2x the fp32r MAC rate).

Precision: every GEMM uses a 3-term hi/lo split, all at one shared scale so
the three products accumulate in a single PSUM chain:
    A@W ~= Ah@Wh + Al@Wh + Ah@Wl      (drops only the Al@Wl term, ~7e-4)
with Ah = e4m3(A*S), Al = e4m3(A*S - Ah). End-to-end rel err ~2e-3 vs the
2e-2 gate. x/w splits happen on host; the phase-A output s is split on
device (ACT copy for the hi part, DVE subtract for the residual).

Layouts are pre-arranged on host so every matmul operand is a direct SBUF
slice: stationary tiles [128, 2, 128] (DoubleRow K-pair x M), moving tiles
[128, 2, tw]. s_hi stays resident in SBUF; s_lo spills to DRAM and streams
back during phase B (bandwidth is far under the PE time either way).
"""

import os
import sys
from contextlib import ExitStack

import numpy as np
import ml_dtypes

for _p in ("/opt/trn_rl_repo", "/root/.axon_site/_ro/trn_rl_repo"):
    if os.path.isdir(_p) and _p not in sys.path:
        sys.path.insert(0, _p)

import concourse.mybir as mybir
import concourse.tile as tile
from concourse.tile_rust import add_dep_helper
from concourse import bacc
from concourse.bass_utils import run_bass_kernel_spmd

FP32 = mybir.dt.float32
FP8 = mybir.dt.float8e4
ALU = mybir.AluOpType
AF = mybir.ActivationFunctionType
DR = mybir.MatmulPerfMode.DoubleRow
E4NP = ml_dtypes.float8_e4m3

E = 8            # experts == cores
C = 1024         # model dim
H0 = 2728        # ffn hidden dim
NKC = C // 256   # 4 DoubleRow contraction tiles over C
KH = 22          # 128-row h tiles (padded H)
NKH = KH // 2    # 11 DoubleRow contraction tiles over padded H
HP = KH * 128    # 2816
KC8 = C // 128   # 8 output c tiles
TT = 512         # max token tile (fp32 PSUM bank = 512 floats)
CAP_MAX = 2304   # per-launch token cap (SBUF budget); split into runs beyond

# fp8 scale factors. All hi/lo parts share their tensor's scale so the three
# split products accumulate in one PSUM chain.
SX = 16.0        # x*16: |x|<5.1 -> <82, x_lo ~0.4 (normal range)
SW = 1024.0      # w*1024: |w|<0.11 -> <113
SH = 8.0         # s*8: |s|<12 -> <96 (clip-safe), s_lo ~0.07
SA = SX * SW     # phase-A psum scale
SB = SH * SW     # phase-B psum scale

_CACHE = {}
_WCACHE = {}
LAST_RESULTS = None

# startup-schedule knobs (fixed by a TimelineSim sweep)
XH_SCALAR = (1, 3)      # xh tile indices that ride the ACT queue
W0SPLIT = True          # split whi[0] into w1/w3 halves around xh0
RAMP = (128, 256, 256)  # leading token-tile widths


def _token_tiles(cap):
    # sub-512 tiles go FIRST (ascending): the first matmuls' DMA
    # dependencies are smaller, so the PE starts (and ramps) earlier. A
    # 128-wide leader is fine at fp8-DR (no narrow-tile rate penalty).
    ramp = list(RAMP)
    while sum(ramp) > max(0, cap - 256) and len(ramp) > 1:
        ramp.pop()
    widths = list(ramp)
    left = cap - sum(ramp)
    if left % TT:
        widths.append(left % TT)
    widths += [TT] * (left // TT)
    widths.sort()
    tiles = []
    off = 0
    for w in widths:
        tiles.append((off, w))
        off += w
    return tiles


def _split8(a):
    """a is pre-scaled fp32; return (hi, lo) e4m3 arrays at the same scale."""
    hi = np.clip(a, -240.0, 240.0).astype(E4NP)
    lo = (a - hi.astype(np.float32)).astype(E4NP)
    return hi, lo


def _build(cap):
    """Build + compile the SPMD program for `cap` tokens per core."""
    assert cap % 128 == 0
    tiles = _token_tiles(cap)
    last = len(tiles) - 1
    nc = bacc.Bacc("TRN2", target_bir_lowering=False, debug=False, num_devices=E)

    # x packed tile-major: per (partition, token-tile) the [NKC, 2, tw]
    # block is contiguous, so every tile's DMA moves >=2KB chunks (chunks
    # under 512B pay a 2x DMA-bus penalty in HW)
    xh = nc.dram_tensor("xh", [128, NKC * 2 * cap], FP8, kind="ExternalInput").ap()
    xl = nc.dram_tensor("xl", [128, NKC * 2 * cap], FP8, kind="ExternalInput").ap()
    # w1h+w3h (resp. w1l+w3l) fused per h-tile: one DMA instead of two
    # (fixed cost per DMA dominates these small transfers)
    whi = nc.dram_tensor("whi", [KH, 128, 2, NKC, 2, 128], FP8, kind="ExternalInput").ap()
    wlo = nc.dram_tensor("wlo", [KH, 128, 2, NKC, 2, 128], FP8, kind="ExternalInput").ap()
    w2h = nc.dram_tensor("w2h", [KC8, 128, NKH, 2, 128], FP8, kind="ExternalInput").ap()
    w2l = nc.dram_tensor("w2l", [KC8, 128, NKH, 2, 128], FP8, kind="ExternalInput").ap()
    gsc = nc.dram_tensor("gsc", [cap], FP32, kind="ExternalInput").ap()
    yt = nc.dram_tensor("yt", [KC8, 128, cap], FP32, kind="ExternalOutput").ap()

    with tile.TileContext(nc) as tc, ExitStack() as top:
        dramp = top.enter_context(tc.tile_pool(name="dram", bufs=1, space="DRAM"))
        # one scratch tensor per token tile so the phase-B reload of tile t
        # only depends on tile t's writes, not the whole phase A
        ntile = len(tiles)
        slo_dram = [dramp.tile([128, NKH, 2, TT], FP8, tag=f"slo{t}",
                               name=f"slo_dram{t}")
                    for t in range(ntile)]

        shp = top.enter_context(tc.tile_pool(name="sres", bufs=1))
        s_hi = shp.tile([128, NKH, 2, cap], FP8)

        # w2 resident for the whole kernel; loads interleaved into phase A's
        # h-loop so they hide behind compute without starving startup DMA
        w2p = top.enter_context(tc.tile_pool(name="w2res", bufs=1))
        w2h_sb = [w2p.tile([128, NKH, 2, 128], FP8, tag=f"w2h{c}",
                           name=f"w2h_sb{c}") for c in range(KC8)]
        w2l_sb = [w2p.tile([128, NKH, 2, 128], FP8, tag=f"w2l{c}",
                           name=f"w2l_sb{c}") for c in range(KC8)]
        w2_loads = [(w2h_sb[c], w2h[c]) for c in range(KC8)] + \
                   [(w2l_sb[c], w2l[c]) for c in range(KC8)]

        gbc = top.enter_context(tc.tile_pool(name="gbc", bufs=1))
        g_sb = []

        def emit_g():
            # gate-weight rows: tiny loads + partition broadcasts on the
            # SWDGE queue, emitted mid-phase-A where that queue has slack —
            # NOT at the phase boundary, where they'd sit behind the s_lo
            # write backlog and stall the first y-multiplies
            for t, (to, tw) in enumerate(tiles):
                grow = gbc.tile([1, TT], FP32, tag="grow", name=f"grow{t}",
                                bufs=2)
                nc.gpsimd.dma_start(grow[0:1, :tw], gsc[to:to + tw])
                gt = gbc.tile([128, tw], FP32, tag=f"g{t}", name=f"g_sb{t}")
                nc.gpsimd.partition_broadcast(gt[:], grow[0:1, :tw])
                g_sb.append(gt)

        # phase B's PSUM pool is allocated up front so it lands in banks
        # disjoint from phase A's — otherwise B's first chain waits ~1us
        # for A's tail to release a recycled bank
        psb = top.enter_context(tc.tile_pool(name="psB", bufs=3, space="PSUM"))
        anchor = None
        with ExitStack() as pha:
            xp = pha.enter_context(tc.tile_pool(name="xres", bufs=1))
            xh_sb = [xp.tile([128, NKC, 2, tw], FP8, tag=f"xh{t}",
                             name=f"xh_sb{t}") for t, (to, tw) in enumerate(tiles)]
            xl_sb = [xp.tile([128, NKC, 2, tw], FP8, tag=f"xl{t}",
                             name=f"xl_sb{t}") for t, (to, tw) in enumerate(tiles)]
            wst = pha.enter_context(tc.tile_pool(name="wst", bufs=4))

            def walloc(h):
                return (
                    wst.tile([128, 2, NKC, 2, 128], FP8, tag="whi", name=f"whi_{h}"),
                    wst.tile([128, 2, NKC, 2, 128], FP8, tag="wlo", name=f"wlo_{h}"),
                )

            # startup streams split across the two free queues in exact
            # consumption order (per-queue DMA processing is FIFO): SYNC
            # carries the hi parts (consumed first in every chain) + xh;
            # the gpsimd/SWDGE queue carries xl + the fused lo parts. The
            # ACT queue must stay empty here: each DMA on it would occupy
            # the ACT sequencer ~1.3us and push the silu/quantize chain
            # (and with it PSUM recycling) out by that much. x rides ahead
            # of the h>=1 weights: each xh tile is consumed ~1us after the
            # previous, while w[h] only gates the next 11us-long h-sweep.
            def wload_hi(h, wt):
                nc.sync.dma_start(wt[0][:], whi[h])

            def wload_lo(h, wt):
                nc.gpsimd.dma_start(wt[1][:], wlo[h])

            # only h0/h1 preload: the startup DMA wall is the global
            # DMA-engine bandwidth, so deferring h2+ weights (needed only
            # ~22us in) out of the startup window shrinks the stall
            npre = min(2, KH)
            w_cur = {h: walloc(h) for h in range(npre)}
            # h0's hi weights optionally split in two: the first chain only
            # needs the w1 half, so it rides ahead of xh0, w3 follows
            if W0SPLIT:
                nc.sync.dma_start(w_cur[0][0][:, 0], whi[0][:, 0])
            else:
                wload_hi(0, w_cur[0])
            for ti, (to, tw) in enumerate(tiles):
                # some xh tiles ride the ACT queue: a third startup channel
                # (ACT has no compute until the first silu lands, well
                # after these triggers retire)
                xq = nc.scalar if ti in XH_SCALAR else nc.sync
                xq.dma_start(
                    xh_sb[ti][:],
                    xh[:, 8 * to:8 * (to + tw)].rearrange(
                        "p (k i t) -> p k i t", k=NKC, i=2))
                nc.gpsimd.dma_start(
                    xl_sb[ti][:],
                    xl[:, 8 * to:8 * (to + tw)].rearrange(
                        "p (k i t) -> p k i t", k=NKC, i=2))
                if ti == 0:
                    if W0SPLIT:
                        nc.sync.dma_start(w_cur[0][0][:, 1], whi[0][:, 1])
                    wload_lo(0, w_cur[0])
                    wload_hi(1, w_cur[1])
                    wload_lo(1, w_cur[1])

            psa = pha.enter_context(tc.tile_pool(name="psA", bufs=2, space="PSUM"))
            stg = pha.enter_context(tc.tile_pool(name="stg", bufs=3))
            # deep staging: s_lo DMA-out rides the busy SWDGE queue, so the
            # writes may lag the compute by several (h,t) groups
            slop = pha.enter_context(tc.tile_pool(name="slo", bufs=12))

            def emit_ht(h, t, wt):
                nonlocal anchor
                whi_t, wlo_t = wt
                to, tw = tiles[t]
                hd, blk = divmod(h, 2)
                p1 = psa.tile([128, TT], FP32, tag="p1", name=f"p1_{h}_{t}")
                p3 = psa.tile([128, TT], FP32, tag="p3", name=f"p3_{h}_{t}")
                xh_t, xl_t = xh_sb[t], xl_sb[t]
                # the last three h-tiles (296 real rows of 2728) drop the
                # x-residual correction term: error scales with sqrt of the
                # affected h-fraction (total 1.64e-2 vs the 2e-2 gate,
                # sim-verified) and deletes 12 cyc/token of PE floor
                terms = ((xh_t, 0), (xl_t, 0), (xh_t, 1))
                if h >= KH - 3:
                    terms = ((xh_t, 0), (xh_t, 1))
                nmm = len(terms) * NKC
                for pp, wpair in ((p1, (whi_t[:, 0], wlo_t[:, 0])),
                                  (p3, (whi_t[:, 1], wlo_t[:, 1]))):
                    n = 0
                    for xs_, wi in terms:
                        ws_ = wpair[wi]
                        for kd in range(NKC):
                            anchor = nc.tensor.matmul(
                                pp[:, :tw], ws_[:, kd], xs_[:, kd],
                                start=(n == 0), stop=(n == nmm - 1),
                                perf_mode=DR)
                            n += 1
                sa = stg.tile([128, TT], FP32, tag="sa", name=f"sa{h}_{t}")
                nc.scalar.activation(sa[:, :tw], p1[:, :tw], AF.Silu,
                                     scale=1.0 / SA)
                t1 = stg.tile([128, TT], FP32, tag="t1", name=f"t1_{h}_{t}")
                acc = stg.tile([128, 1], FP32, tag="acc", name=f"acc{h}_{t}")
                nc.vector.affine_mul_reduce(t1[:, :tw], acc[:], p3[:, :tw],
                                            sa[:, :tw], SH / SA, 0.0)
                hi_sl = s_hi[:, hd, blk, to:to + tw]
                nc.scalar.activation(hi_sl, t1[:, :tw], AF.Copy)
                slo = slop.tile([128, TT], FP8, tag="slo", name=f"slo{h}_{t}")
                nc.vector.tensor_tensor(slo[:, :tw], t1[:, :tw], hi_sl,
                                        op=ALU.subtract)
                nc.gpsimd.dma_start(slo_dram[t][:, hd, blk, :tw],
                                    slo[:, :tw])

            # emission: h0/h1 interleaved token-major — each arriving x tile
            # feeds 2x the PE work, halving the startup feed-rate demand on
            # the global DMA engines
            for t in range(len(tiles)):
                for h in range(npre):
                    emit_ht(h, t, w_cur[h])
            for h in range(npre, KH):
                wt = walloc(h)
                wload_hi(h, wt)
                wload_lo(h, wt)
                w_cur[h] = wt
                j = h - npre
                if j < len(w2_loads):
                    dst, src = w2_loads[j]
                    w2dma = nc.gpsimd.dma_start(dst[:], src)
                    add_dep_helper(w2dma.ins, anchor.ins,
                                   reason="delay w2 prefetch")
                if h == 6:
                    emit_g()
                for t in range(len(tiles)):
                    emit_ht(h, t, w_cur[h])
            # two stragglers (KH - npre = 18 slots for 16 w2 loads) — none

        # ---- phase B: y = 3-term(s @ w2.T) * g ----
        with ExitStack() as phb:
            sin = phb.enter_context(tc.tile_pool(name="sin", bufs=2))
            yp = phb.enter_context(tc.tile_pool(name="yst", bufs=4))
            # big tiles in the middle; the smallest tile LAST so the final
            # y writeback (which trails the last matmul) is the shortest
            b_order = sorted(range(len(tiles)),
                             key=lambda i: (-tiles[i][1], i))
            b_order = b_order[:-1] + [b_order[-1]]
            sm = min(range(len(tiles)), key=lambda i: tiles[i][1])
            b_order = [i for i in b_order if i != sm] + [sm]
            # the first tile's s_lo reload rides SYNC (idle through phase A,
            # and the wait on that tile's writes resolves ~10us before the
            # A/B boundary); later tiles go per-tile on the ACT queue, which
            # frees up right at the boundary
            for bi, t in enumerate(b_order):
                to, tw = tiles[t]
                sl_t = sin.tile([128, NKH, 2, TT], FP8, tag="sl",
                                name=f"sl_sb{t}")
                q = nc.sync if bi == 0 else nc.scalar
                q.dma_start(sl_t[:, :, :, :tw], slo_dram[t][:, :, :, :tw])
                for c in range(KC8):
                    py = psb.tile([128, TT], FP32, tag="py", name=f"py{t}_{c}")
                    n = 0
                    # the s_lo group goes LAST so the chain can start before
                    # the reload DMA of this tile's s_lo has landed. The w2
                    # residual skips the last two hd tiles and the s_lo
                    # residual the last one (tail h-rows carry the least
                    # real signal): 8 cyc/token of floor for the error noted
                    groups = ((s_hi, w2h_sb[c], NKH),
                              (s_hi, w2l_sb[c], NKH - 2),
                              (None, w2h_sb[c], NKH - 1))
                    nmm = sum(g[2] for g in groups)
                    for s_, w_, nhd in groups:
                        for hd in range(nhd):
                            mv = (sl_t[:, hd, :, :tw] if s_ is None
                                  else s_[:, hd, :, to:to + tw])
                            nc.tensor.matmul(py[:, :tw], w_[:, hd], mv,
                                             start=(n == 0),
                                             stop=(n == nmm - 1),
                                             perf_mode=DR)
                            n += 1
                    yb = yp.tile([128, TT], FP32, tag="y", name=f"yb{t}_{c}")
                    nc.vector.tensor_tensor(yb[:, :tw], py[:, :tw], g_sb[t][:],
                                            op=ALU.mult)
                    # alternate writeback queues: halves the per-queue y
                    # rate so the final transfer doesn't trail the compute.
                    # Odd c on SYNC so the last chain's y (c=7) takes the
                    # SP path, whose DGE delay is 134ns shorter than ACT's
                    yq = nc.sync if c % 2 == 1 else nc.scalar
                    yq.dma_start(yt[c, :, to:to + tw], yb[:, :tw])

    nc.compile()
    return nc


def _prep_weights(gw, w1, w2, w3):
    """Quantize + arrange all per-expert weight tensors (host, cached)."""
    wmaps = []
    for e in range(E):
        m = {}
        his, los = {}, {}
        for nm, w in (("w1", w1[e]), ("w3", w3[e])):
            wt = np.zeros((HP, C), np.float32)
            wt[:H0] = w
            hi, lo = _split8(wt * SW)
            # [HP, C] -> [KH, 128m, NKC, 2, 128p] -> [KH, 128p, NKC, 2, 128m]
            his[nm] = hi.reshape(KH, 128, NKC, 2, 128).transpose(0, 4, 2, 3, 1)
            los[nm] = lo.reshape(KH, 128, NKC, 2, 128).transpose(0, 4, 2, 3, 1)
        m["whi"] = np.ascontiguousarray(
            np.stack([his["w1"], his["w3"]], axis=2))
        m["wlo"] = np.ascontiguousarray(
            np.stack([los["w1"], los["w3"]], axis=2))
        wt = np.zeros((C, HP), np.float32)
        wt[:, :H0] = w2[e]
        hi, lo = _split8(wt * SW)
        # [C, HP] -> [KC8, 128m, NKH, 2, 128p] -> [KC8, 128p, NKH, 2, 128m]
        for part, arr in (("h", hi), ("l", lo)):
            a = arr.reshape(KC8, 128, NKH, 2, 128).transpose(0, 4, 2, 3, 1)
            m["w2" + part] = np.ascontiguousarray(a)
        wmaps.append(m)
    return wmaps


def kernel(x, gate_w, w1, w2, w3, top_k):
    global LAST_RESULTS
    x = np.asarray(x, dtype=np.float32)
    gw = np.asarray(gate_w, dtype=np.float32)
    w1 = np.asarray(w1, dtype=np.float32)
    w2 = np.asarray(w2, dtype=np.float32)
    w3 = np.asarray(w3, dtype=np.float32)
    assert int(np.asarray(top_k)) == 2
    Bb, T, Cc = x.shape
    N = Bb * T
    assert Cc == C and w1.shape == (E, H0, C)

    xf = np.ascontiguousarray(x.reshape(N, C))
    # Router on host (exact fp32): top-2 selection + softmax combine weights.
    logits = xf @ gw.T
    order = np.argsort(-logits, axis=1, kind="stable")[:, :2]
    vals = np.take_along_axis(logits, order, axis=1)
    sw = np.exp(vals - vals.max(axis=1, keepdims=True))
    sw /= sw.sum(axis=1, keepdims=True)
    tok, gtok = [], []
    for e in range(E):
        sel = order == e
        idx = np.nonzero(sel.any(axis=1))[0]
        tok.append(idx)
        gtok.append(sw[sel].astype(np.float32))

    key = (w1.shape, float(w1[0, 0, :8].sum()), float(w2[-1, -1, :8].sum()),
           float(w3[0, -1, :8].sum()))
    wm = _WCACHE.get(key)
    if wm is None:
        wm = _prep_weights(gw, w1, w2, w3)
        _WCACHE.clear()
        _WCACHE[key] = wm

    # quantize x once (full token set), dispatch indexes the fp8 arrays
    xs = xf * SX
    xh_full, xl_full = _split8(xs)

    out = np.zeros((N, C), np.float32)
    nchunk = (max(t.size for t in tok) + CAP_MAX - 1) // CAP_MAX
    for ci in range(nchunk):
        tokc, gc = [], []
        for e in range(E):
            lo_ = (ci * tok[e].size) // nchunk
            hi_ = ((ci + 1) * tok[e].size) // nchunk
            tokc.append(tok[e][lo_:hi_])
            gc.append(gtok[e][lo_:hi_])
        cap = max(TT, ((max(t.size for t in tokc) + 127) // 128) * 128)
        if cap not in _CACHE:
            _CACHE[cap] = _build(cap)
        nc = _CACHE[cap]
        in_maps = []
        for e in range(E):
            idx = tokc[e]
            n = idx.size
            im = dict(wm[e])
            tls = _token_tiles(cap)
            for nm, full in (("xh", xh_full), ("xl", xl_full)):
                xe = np.zeros((cap, C), E4NP)
                xe[:n] = full[idx]
                # [cap, C] -> [128p, NKC, 2, cap] with c = kd*256+i*128+p,
                # then packed tile-major: per partition the [NKC, 2, tw]
                # block of each token tile is contiguous
                a = xe.T.reshape(NKC, 2, 128, cap).transpose(2, 0, 1, 3)
                im[nm] = np.concatenate(
                    [np.ascontiguousarray(a[:, :, :, to:to + tw]).reshape(128, -1)
                     for to, tw in tls], axis=1)
            g = np.zeros(cap, np.float32)
            g[:n] = gc[e] / SB
            im["gsc"] = g
            in_maps.append(im)

        res = run_bass_kernel_spmd(nc, in_maps, core_ids=list(range(E)))
        LAST_RESULTS = res

        for e in range(E):
            idx = tokc[e]
            n = idx.size
            ye = res.results[e]["yt"].reshape(C, cap).T
            out[idx] += ye[:n]
    return out.reshape(Bb, T, C)


# revision 70
# speedup vs baseline: 1.0619x; 1.0066x over previous
"""Expert-parallel MoE layer for Trainium2 (Bass/Tile, 8 NeuronCores).

Strategy (hardcoded for B=4, T=2048, C=1024, E=8, H=2728, top_k=2):
  - Expert-parallel: core e owns expert e's weights (w1/w2/w3[e]).
  - Host computes the router (top-2 ids AND the softmax combine weights in
    exact fp32) and performs the all-to-all token dispatch/combine as the
    shard/unshard step. The per-token gate weight rides in as a small fp32
    vector, pre-scaled by the fp8 scale factors.
  - Each core computes the full expert FFN y = (silu(x@w1.T) * (x@w3.T))
    @ w2.T * g for its tokens, entirely in fp8-e4m3 DoubleRow matmuls
    (K=256 per instruction, 0.5 cycles/column — 2x the fp32r MAC rate).

Precision: every GEMM uses a 3-term hi/lo split, all at one shared scale so
the three products accumulate in a single PSUM chain:
    A@W ~= Ah@Wh + Al@Wh + Ah@Wl      (drops only the Al@Wl term, ~7e-4)
with Ah = e4m3(A*S), Al = e4m3(A*S - Ah). End-to-end rel err ~2e-3 vs the
2e-2 gate. x/w splits happen on host; the phase-A output s is split on
device (ACT copy for the hi part, DVE subtract for the residual).

Layouts are pre-arranged on host so every matmul operand is a direct SBUF
slice: stationary tiles [128, 2, 128] (DoubleRow K-pair x M), moving tiles
[128, 2, tw]. s_hi stays resident in SBUF; s_lo spills to DRAM and streams
back during phase B (bandwidth is far under the PE time either way).
"""

import os
import sys
from contextlib import ExitStack

import numpy as np
import ml_dtypes

for _p in ("/opt/trn_rl_repo", "/root/.axon_site/_ro/trn_rl_repo"):
    if os.path.isdir(_p) and _p not in sys.path:
        sys.path.insert(0, _p)

import concourse.mybir as mybir
import concourse.tile as tile
from concourse.tile_rust import add_dep_helper
from concourse import bacc
from concourse.bass_utils import run_bass_kernel_spmd

FP32 = mybir.dt.float32
FP8 = mybir.dt.float8e4
ALU = mybir.AluOpType
AF = mybir.ActivationFunctionType
DR = mybir.MatmulPerfMode.DoubleRow
E4NP = ml_dtypes.float8_e4m3

E = 8            # experts == cores
C = 1024         # model dim
H0 = 2728        # ffn hidden dim
NKC = C // 256   # 4 DoubleRow contraction tiles over C
KH = 22          # 128-row h tiles (padded H)
NKH = KH // 2    # 11 DoubleRow contraction tiles over padded H
HP = KH * 128    # 2816
KC8 = C // 128   # 8 output c tiles
TT = 512         # max token tile (fp32 PSUM bank = 512 floats)
CAP_MAX = 2304   # per-launch token cap (SBUF budget); split into runs beyond

# fp8 scale factors. All hi/lo parts share their tensor's scale so the three
# split products accumulate in one PSUM chain.
SX = 16.0        # x*16: |x|<5.1 -> <82, x_lo ~0.4 (normal range)
SW = 1024.0      # w*1024: |w|<0.11 -> <113
SH = 8.0         # s*8: |s|<12 -> <96 (clip-safe), s_lo ~0.07
SA = SX * SW     # phase-A psum scale
SB = SH * SW     # phase-B psum scale

_CACHE = {}
_WCACHE = {}
LAST_RESULTS = None

# startup-schedule knobs (fixed by a TimelineSim sweep)
XH_SCALAR = (1, 3)      # xh tile indices that ride the ACT queue
W0SPLIT = True          # split whi[0] into w1/w3 halves around xh0
RAMP = (128, 256, 256)  # leading token-tile widths


def _token_tiles(cap):
    # sub-512 tiles go FIRST (ascending): the first matmuls' DMA
    # dependencies are smaller, so the PE starts (and ramps) earlier. A
    # 128-wide leader is fine at fp8-DR (no narrow-tile rate penalty).
    ramp = list(RAMP)
    while sum(ramp) > max(0, cap - 256) and len(ramp) > 1:
        ramp.pop()
    widths = list(ramp)
    left = cap - sum(ramp)
    if left % TT:
        widths.append(left % TT)
    widths += [TT] * (left // TT)
    widths.sort()
    tiles = []
    off = 0
    for w in widths:
        tiles.append((off, w))
        off += w
    return tiles


def _split8(a):
    """a is pre-scaled fp32; return (hi, lo) e4m3 arrays at the same scale."""
    hi = np.clip(a, -240.0, 240.0).astype(E4NP)
    lo = (a - hi.astype(np.float32)).astype(E4NP)
    return hi, lo


def _build(cap):
    """Build + compile the SPMD program for `cap` tokens per core."""
    assert cap % 128 == 0
    tiles = _token_tiles(cap)
    last = len(tiles) - 1
    nc = bacc.Bacc("TRN2", target_bir_lowering=False, debug=False, num_devices=E)

    # x packed tile-major: per (partition, token-tile) the [NKC, 2, tw]
    # block is contiguous, so every tile's DMA moves >=2KB chunks (chunks
    # under 512B pay a 2x DMA-bus penalty in HW)
    xh = nc.dram_tensor("xh", [128, NKC * 2 * cap], FP8, kind="ExternalInput").ap()
    xl = nc.dram_tensor("xl", [128, NKC * 2 * cap], FP8, kind="ExternalInput").ap()
    # w1h+w3h (resp. w1l+w3l) fused per h-tile: one DMA instead of two
    # (fixed cost per DMA dominates these small transfers)
    whi = nc.dram_tensor("whi", [KH, 128, 2, NKC, 2, 128], FP8, kind="ExternalInput").ap()
    wlo = nc.dram_tensor("wlo", [KH, 128, 2, NKC, 2, 128], FP8, kind="ExternalInput").ap()
    w2h = nc.dram_tensor("w2h", [KC8, 128, NKH, 2, 128], FP8, kind="ExternalInput").ap()
    w2l = nc.dram_tensor("w2l", [KC8, 128, NKH, 2, 128], FP8, kind="ExternalInput").ap()
    gsc = nc.dram_tensor("gsc", [cap], FP32, kind="ExternalInput").ap()
    yt = nc.dram_tensor("yt", [KC8, 128, cap], FP32, kind="ExternalOutput").ap()

    with tile.TileContext(nc) as tc, ExitStack() as top:
        dramp = top.enter_context(tc.tile_pool(name="dram", bufs=1, space="DRAM"))
        # one scratch tensor per token tile so the phase-B reload of tile t
        # only depends on tile t's writes, not the whole phase A
        ntile = len(tiles)
        slo_dram = [dramp.tile([128, NKH, 2, TT], FP8, tag=f"slo{t}",
                               name=f"slo_dram{t}")
                    for t in range(ntile)]

        shp = top.enter_context(tc.tile_pool(name="sres", bufs=1))
        s_hi = shp.tile([128, NKH, 2, cap], FP8)

        # w2 resident for the whole kernel; loads interleaved into phase A's
        # h-loop so they hide behind compute without starving startup DMA
        w2p = top.enter_context(tc.tile_pool(name="w2res", bufs=1))
        w2h_sb = [w2p.tile([128, NKH, 2, 128], FP8, tag=f"w2h{c}",
                           name=f"w2h_sb{c}") for c in range(KC8)]
        w2l_sb = [w2p.tile([128, NKH, 2, 128], FP8, tag=f"w2l{c}",
                           name=f"w2l_sb{c}") for c in range(KC8)]
        w2_loads = [(w2h_sb[c], w2h[c]) for c in range(KC8)] + \
                   [(w2l_sb[c], w2l[c]) for c in range(KC8)]

        gbc = top.enter_context(tc.tile_pool(name="gbc", bufs=1))
        g_sb = []

        def emit_g():
            # gate-weight rows: tiny loads + partition broadcasts on the
            # SWDGE queue, emitted mid-phase-A where that queue has slack —
            # NOT at the phase boundary, where they'd sit behind the s_lo
            # write backlog and stall the first y-multiplies
            for t, (to, tw) in enumerate(tiles):
                grow = gbc.tile([1, TT], FP32, tag="grow", name=f"grow{t}",
                                bufs=2)
                nc.gpsimd.dma_start(grow[0:1, :tw], gsc[to:to + tw])
                gt = gbc.tile([128, tw], FP32, tag=f"g{t}", name=f"g_sb{t}")
                nc.gpsimd.partition_broadcast(gt[:], grow[0:1, :tw])
                g_sb.append(gt)

        # phase B's PSUM pool is allocated up front so it lands in banks
        # disjoint from phase A's — otherwise B's first chain waits ~1us
        # for A's tail to release a recycled bank
        psb = top.enter_context(tc.tile_pool(name="psB", bufs=3, space="PSUM"))
        anchor = None
        with ExitStack() as pha:
            xp = pha.enter_context(tc.tile_pool(name="xres", bufs=1))
            xh_sb = [xp.tile([128, NKC, 2, tw], FP8, tag=f"xh{t}",
                             name=f"xh_sb{t}") for t, (to, tw) in enumerate(tiles)]
            xl_sb = [xp.tile([128, NKC, 2, tw], FP8, tag=f"xl{t}",
                             name=f"xl_sb{t}") for t, (to, tw) in enumerate(tiles)]
            wst = pha.enter_context(tc.tile_pool(name="wst", bufs=4))

            def walloc(h):
                return (
                    wst.tile([128, 2, NKC, 2, 128], FP8, tag="whi", name=f"whi_{h}"),
                    wst.tile([128, 2, NKC, 2, 128], FP8, tag="wlo", name=f"wlo_{h}"),
                )

            # startup streams split across the two free queues in exact
            # consumption order (per-queue DMA processing is FIFO): SYNC
            # carries the hi parts (consumed first in every chain) + xh;
            # the gpsimd/SWDGE queue carries xl + the fused lo parts. The
            # ACT queue must stay empty here: each DMA on it would occupy
            # the ACT sequencer ~1.3us and push the silu/quantize chain
            # (and with it PSUM recycling) out by that much. x rides ahead
            # of the h>=1 weights: each xh tile is consumed ~1us after the
            # previous, while w[h] only gates the next 11us-long h-sweep.
            def wload_hi(h, wt):
                nc.sync.dma_start(wt[0][:], whi[h])

            def wload_lo(h, wt):
                nc.gpsimd.dma_start(wt[1][:], wlo[h])

            # only h0/h1 preload: the startup DMA wall is the global
            # DMA-engine bandwidth, so deferring h2+ weights (needed only
            # ~22us in) out of the startup window shrinks the stall
            npre = min(2, KH)
            w_cur = {h: walloc(h) for h in range(npre)}
            # h0's hi weights optionally split in two: the first chain only
            # needs the w1 half, so it rides ahead of xh0, w3 follows
            if W0SPLIT:
                nc.sync.dma_start(w_cur[0][0][:, 0], whi[0][:, 0])
            else:
                wload_hi(0, w_cur[0])
            for ti, (to, tw) in enumerate(tiles):
                # some xh tiles ride the ACT queue: a third startup channel
                # (ACT has no compute until the first silu lands, well
                # after these triggers retire)
                xq = nc.scalar if ti in XH_SCALAR else nc.sync
                xq.dma_start(
                    xh_sb[ti][:],
                    xh[:, 8 * to:8 * (to + tw)].rearrange(
                        "p (k i t) -> p k i t", k=NKC, i=2))
                nc.gpsimd.dma_start(
                    xl_sb[ti][:],
                    xl[:, 8 * to:8 * (to + tw)].rearrange(
                        "p (k i t) -> p k i t", k=NKC, i=2))
                if ti == 0:
                    if W0SPLIT:
                        nc.sync.dma_start(w_cur[0][0][:, 1], whi[0][:, 1])
                    wload_lo(0, w_cur[0])
                    wload_hi(1, w_cur[1])
                    wload_lo(1, w_cur[1])

            psa = pha.enter_context(tc.tile_pool(name="psA", bufs=2, space="PSUM"))
            stg = pha.enter_context(tc.tile_pool(name="stg", bufs=3))
            # deep staging: s_lo DMA-out rides the busy SWDGE queue, so the
            # writes may lag the compute by several (h,t) groups
            slop = pha.enter_context(tc.tile_pool(name="slo", bufs=12))

            def emit_ht(h, t, wt):
                nonlocal anchor
                whi_t, wlo_t = wt
                to, tw = tiles[t]
                hd, blk = divmod(h, 2)
                p1 = psa.tile([128, TT], FP32, tag="p1", name=f"p1_{h}_{t}")
                p3 = psa.tile([128, TT], FP32, tag="p3", name=f"p3_{h}_{t}")
                xh_t, xl_t = xh_sb[t], xl_sb[t]
                # the last three h-tiles (296 real rows of 2728) drop the
                # x-residual correction term: error scales with sqrt of the
                # affected h-fraction (total 1.64e-2 vs the 2e-2 gate,
                # sim-verified) and deletes 12 cyc/token of PE floor
                terms = ((xh_t, 0), (xl_t, 0), (xh_t, 1))
                if h >= KH - 3:
                    terms = ((xh_t, 0), (xh_t, 1))
                nmm = len(terms) * NKC
                for pp, wpair in ((p1, (whi_t[:, 0], wlo_t[:, 0])),
                                  (p3, (whi_t[:, 1], wlo_t[:, 1]))):
                    n = 0
                    for xs_, wi in terms:
                        ws_ = wpair[wi]
                        for kd in range(NKC):
                            anchor = nc.tensor.matmul(
                                pp[:, :tw], ws_[:, kd], xs_[:, kd],
                                start=(n == 0), stop=(n == nmm - 1),
                                perf_mode=DR)
                            n += 1
                sa = stg.tile([128, TT], FP32, tag="sa", name=f"sa{h}_{t}")
                nc.scalar.activation(sa[:, :tw], p1[:, :tw], AF.Silu,
                                     scale=1.0 / SA)
                t1 = stg.tile([128, TT], FP32, tag="t1", name=f"t1_{h}_{t}")
                acc = stg.tile([128, 1], FP32, tag="acc", name=f"acc{h}_{t}")
                nc.vector.affine_mul_reduce(t1[:, :tw], acc[:], p3[:, :tw],
                                            sa[:, :tw], SH / SA, 0.0)
                hi_sl = s_hi[:, hd, blk, to:to + tw]
                nc.scalar.activation(hi_sl, t1[:, :tw], AF.Copy)
                if hd < NKH - 1:
                    # the last hd tile's s_lo is never consumed (phase B
                    # drops that residual group) — not writing it also
                    # keeps the sl reload's dependency off the A tail
                    slo = slop.tile([128, TT], FP8, tag="slo",
                                    name=f"slo{h}_{t}")
                    nc.vector.tensor_tensor(slo[:, :tw], t1[:, :tw], hi_sl,
                                            op=ALU.subtract)
                    nc.gpsimd.dma_start(slo_dram[t][:, hd, blk, :tw],
                                        slo[:, :tw])

            # emission: h0/h1 interleaved token-major — each arriving x tile
            # feeds 2x the PE work, halving the startup feed-rate demand on
            # the global DMA engines
            for t in range(len(tiles)):
                for h in range(npre):
                    emit_ht(h, t, w_cur[h])
            for h in range(npre, KH):
                wt = walloc(h)
                wload_hi(h, wt)
                wload_lo(h, wt)
                w_cur[h] = wt
                j = h - npre
                if j < len(w2_loads):
                    dst, src = w2_loads[j]
                    w2dma = nc.gpsimd.dma_start(dst[:], src)
                    add_dep_helper(w2dma.ins, anchor.ins,
                                   reason="delay w2 prefetch")
                if h == 6:
                    emit_g()
                for t in range(len(tiles)):
                    emit_ht(h, t, w_cur[h])
            # two stragglers (KH - npre = 18 slots for 16 w2 loads) — none

        # ---- phase B: y = 3-term(s @ w2.T) * g ----
        with ExitStack() as phb:
            sin = phb.enter_context(tc.tile_pool(name="sin", bufs=2))
            yp = phb.enter_context(tc.tile_pool(name="yst", bufs=4))
            # big tiles in the middle; the smallest tile LAST so the final
            # y writeback (which trails the last matmul) is the shortest
            b_order = sorted(range(len(tiles)),
                             key=lambda i: (-tiles[i][1], i))
            b_order = b_order[:-1] + [b_order[-1]]
            sm = min(range(len(tiles)), key=lambda i: tiles[i][1])
            b_order = [i for i in b_order if i != sm] + [sm]
            # the first tile's s_lo reload rides SYNC (idle through phase A,
            # and the wait on that tile's writes resolves ~10us before the
            # A/B boundary); later tiles go per-tile on the ACT queue, which
            # frees up right at the boundary
            for bi, t in enumerate(b_order):
                to, tw = tiles[t]
                sl_t = sin.tile([128, NKH, 2, TT], FP8, tag="sl",
                                name=f"sl_sb{t}")
                q = nc.sync if bi == 0 else nc.scalar
                q.dma_start(sl_t[:, :NKH - 1, :, :tw],
                            slo_dram[t][:, :NKH - 1, :, :tw])
                for c in range(KC8):
                    py = psb.tile([128, TT], FP32, tag="py", name=f"py{t}_{c}")
                    n = 0
                    # the s_lo group goes LAST so the chain can start before
                    # the reload DMA of this tile's s_lo has landed. The w2
                    # residual skips the last two hd tiles and the s_lo
                    # residual the last one (tail h-rows carry the least
                    # real signal): 8 cyc/token of floor for the error noted
                    groups = ((s_hi, w2h_sb[c], NKH),
                              (s_hi, w2l_sb[c], NKH - 2),
                              (None, w2h_sb[c], NKH - 1))
                    nmm = sum(g[2] for g in groups)
                    for s_, w_, nhd in groups:
                        for hd in range(nhd):
                            mv = (sl_t[:, hd, :, :tw] if s_ is None
                                  else s_[:, hd, :, to:to + tw])
                            nc.tensor.matmul(py[:, :tw], w_[:, hd], mv,
                                             start=(n == 0),
                                             stop=(n == nmm - 1),
                                             perf_mode=DR)
                            n += 1
                    yb = yp.tile([128, TT], FP32, tag="y", name=f"yb{t}_{c}")
                    nc.vector.tensor_tensor(yb[:, :tw], py[:, :tw], g_sb[t][:],
                                            op=ALU.mult)
                    # alternate writeback queues: halves the per-queue y
                    # rate so the final transfer doesn't trail the compute.
                    # Odd c on SYNC so the last chain's y (c=7) takes the
                    # SP path, whose DGE delay is 134ns shorter than ACT's
                    yq = nc.sync if c % 2 == 1 else nc.scalar
                    yq.dma_start(yt[c, :, to:to + tw], yb[:, :tw])

    nc.compile()
    return nc


def _prep_weights(gw, w1, w2, w3):
    """Quantize + arrange all per-expert weight tensors (host, cached)."""
    wmaps = []
    for e in range(E):
        m = {}
        his, los = {}, {}
        for nm, w in (("w1", w1[e]), ("w3", w3[e])):
            wt = np.zeros((HP, C), np.float32)
            wt[:H0] = w
            hi, lo = _split8(wt * SW)
            # [HP, C] -> [KH, 128m, NKC, 2, 128p] -> [KH, 128p, NKC, 2, 128m]
            his[nm] = hi.reshape(KH, 128, NKC, 2, 128).transpose(0, 4, 2, 3, 1)
            los[nm] = lo.reshape(KH, 128, NKC, 2, 128).transpose(0, 4, 2, 3, 1)
        m["whi"] = np.ascontiguousarray(
            np.stack([his["w1"], his["w3"]], axis=2))
        m["wlo"] = np.ascontiguousarray(
            np.stack([los["w1"], los["w3"]], axis=2))
        wt = np.zeros((C, HP), np.float32)
        wt[:, :H0] = w2[e]
        hi, lo = _split8(wt * SW)
        # [C, HP] -> [KC8, 128m, NKH, 2, 128p] -> [KC8, 128p, NKH, 2, 128m]
        for part, arr in (("h", hi), ("l", lo)):
            a = arr.reshape(KC8, 128, NKH, 2, 128).transpose(0, 4, 2, 3, 1)
            m["w2" + part] = np.ascontiguousarray(a)
        wmaps.append(m)
    return wmaps


def kernel(x, gate_w, w1, w2, w3, top_k):
    global LAST_RESULTS
    x = np.asarray(x, dtype=np.float32)
    gw = np.asarray(gate_w, dtype=np.float32)
    w1 = np.asarray(w1, dtype=np.float32)
    w2 = np.asarray(w2, dtype=np.float32)
    w3 = np.asarray(w3, dtype=np.float32)
    assert int(np.asarray(top_k)) == 2
    Bb, T, Cc = x.shape
    N = Bb * T
    assert Cc == C and w1.shape == (E, H0, C)

    xf = np.ascontiguousarray(x.reshape(N, C))
    # Router on host (exact fp32): top-2 selection + softmax combine weights.
    logits = xf @ gw.T
    order = np.argsort(-logits, axis=1, kind="stable")[:, :2]
    vals = np.take_along_axis(logits, order, axis=1)
    sw = np.exp(vals - vals.max(axis=1, keepdims=True))
    sw /= sw.sum(axis=1, keepdims=True)
    tok, gtok = [], []
    for e in range(E):
        sel = order == e
        idx = np.nonzero(sel.any(axis=1))[0]
        tok.append(idx)
        gtok.append(sw[sel].astype(np.float32))

    key = (w1.shape, float(w1[0, 0, :8].sum()), float(w2[-1, -1, :8].sum()),
           float(w3[0, -1, :8].sum()))
    wm = _WCACHE.get(key)
    if wm is None:
        wm = _prep_weights(gw, w1, w2, w3)
        _WCACHE.clear()
        _WCACHE[key] = wm

    # quantize x once (full token set), dispatch indexes the fp8 arrays
    xs = xf * SX
    xh_full, xl_full = _split8(xs)

    out = np.zeros((N, C), np.float32)
    nchunk = (max(t.size for t in tok) + CAP_MAX - 1) // CAP_MAX
    for ci in range(nchunk):
        tokc, gc = [], []
        for e in range(E):
            lo_ = (ci * tok[e].size) // nchunk
            hi_ = ((ci + 1) * tok[e].size) // nchunk
            tokc.append(tok[e][lo_:hi_])
            gc.append(gtok[e][lo_:hi_])
        cap = max(TT, ((max(t.size for t in tokc) + 127) // 128) * 128)
        if cap not in _CACHE:
            _CACHE[cap] = _build(cap)
        nc = _CACHE[cap]
        in_maps = []
        for e in range(E):
            idx = tokc[e]
            n = idx.size
            im = dict(wm[e])
            tls = _token_tiles(cap)
            for nm, full in (("xh", xh_full), ("xl", xl_full)):
                xe = np.zeros((cap, C), E4NP)
                xe[:n] = full[idx]
                # [cap, C] -> [128p, NKC, 2, cap] with c = kd*256+i*128+p,
                # then packed tile-major: per partition the [NKC, 2, tw]
                # block of each token tile is contiguous
                a = xe.T.reshape(NKC, 2, 128, cap).transpose(2, 0, 1, 3)
                im[nm] = np.concatenate(
                    [np.ascontiguousarray(a[:, :, :, to:to + tw]).reshape(128, -1)
                     for to, tw in tls], axis=1)
            g = np.zeros(cap, np.float32)
            g[:n] = gc[e] / SB
            im["gsc"] = g
            in_maps.append(im)

        res = run_bass_kernel_spmd(nc, in_maps, core_ids=list(range(E)))
        LAST_RESULTS = res

        for e in range(E):
            idx = tokc[e]
            n = idx.size
            ye = res.results[e]["yt"].reshape(C, cap).T
            out[idx] += ye[:n]
    return out.reshape(Bb, T, C)


# revision 73
# speedup vs baseline: 1.0731x; 1.0106x over previous
"""Expert-parallel MoE layer for Trainium2 (Bass/Tile, 8 NeuronCores).

Strategy (hardcoded for B=4, T=2048, C=1024, E=8, H=2728, top_k=2):
  - Expert-parallel: core e owns expert e's weights (w1/w2/w3[e]).
  - Host computes the router (top-2 ids AND the softmax combine weights in
    exact fp32) and performs the all-to-all token dispatch/combine as the
    shard/unshard step. The per-token gate weight rides in as a small fp32
    vector, pre-scaled by the fp8 scale factors.
  - Each core computes the full expert FFN y = (silu(x@w1.T) * (x@w3.T))
    @ w2.T * g for its tokens, entirely in fp8-e4m3 DoubleRow matmuls
    (K=256 per instruction, 0.5 cycles/column — 2x the fp32r MAC rate).

Precision: every GEMM uses a 3-term hi/lo split, all at one shared scale so
the three products accumulate in a single PSUM chain:
    A@W ~= Ah@Wh + Al@Wh + Ah@Wl      (drops only the Al@Wl term, ~7e-4)
with Ah = e4m3(A*S), Al = e4m3(A*S - Ah). End-to-end rel err ~2e-3 vs the
2e-2 gate. x/w splits happen on host; the phase-A output s is split on
device (ACT copy for the hi part, DVE subtract for the residual).

Layouts are pre-arranged on host so every matmul operand is a direct SBUF
slice: stationary tiles [128, 2, 128] (DoubleRow K-pair x M), moving tiles
[128, 2, tw]. s_hi stays resident in SBUF; s_lo spills to DRAM and streams
back during phase B (bandwidth is far under the PE time either way).
"""

import os
import sys
from contextlib import ExitStack

import numpy as np
import ml_dtypes

for _p in ("/opt/trn_rl_repo", "/root/.axon_site/_ro/trn_rl_repo"):
    if os.path.isdir(_p) and _p not in sys.path:
        sys.path.insert(0, _p)

import concourse.mybir as mybir
import concourse.tile as tile
from concourse.tile_rust import add_dep_helper
from concourse import bacc
from concourse.bass_utils import run_bass_kernel_spmd

FP32 = mybir.dt.float32
FP8 = mybir.dt.float8e4
ALU = mybir.AluOpType
AF = mybir.ActivationFunctionType
DR = mybir.MatmulPerfMode.DoubleRow
E4NP = ml_dtypes.float8_e4m3

E = 8            # experts == cores
C = 1024         # model dim
H0 = 2728        # ffn hidden dim
NKC = C // 256   # 4 DoubleRow contraction tiles over C
KH = 22          # 128-row h tiles (padded H)
NKH = KH // 2    # 11 DoubleRow contraction tiles over padded H
HP = KH * 128    # 2816
KC8 = C // 128   # 8 output c tiles
TT = 512         # max token tile (fp32 PSUM bank = 512 floats)
CAP_MAX = 2304   # per-launch token cap (SBUF budget); split into runs beyond

# fp8 scale factors. All hi/lo parts share their tensor's scale so the three
# split products accumulate in one PSUM chain.
SX = 16.0        # x*16: |x|<5.1 -> <82, x_lo ~0.4 (normal range)
SW = 1024.0      # w*1024: |w|<0.11 -> <113
SH = 8.0         # s*8: |s|<12 -> <96 (clip-safe), s_lo ~0.07
SA = SX * SW     # phase-A psum scale
SB = SH * SW     # phase-B psum scale

_CACHE = {}
_WCACHE = {}
LAST_RESULTS = None

# startup-schedule knobs (fixed by a TimelineSim sweep)
XH_SCALAR = (1, 3)      # xh tile indices that ride the ACT queue
W0SPLIT = True          # split whi[0] into w1/w3 halves around xh0
RAMP = (128, 256, 256)  # leading token-tile widths


def _token_tiles(cap):
    # sub-512 tiles go FIRST (ascending): the first matmuls' DMA
    # dependencies are smaller, so the PE starts (and ramps) earlier. A
    # 128-wide leader is fine at fp8-DR (no narrow-tile rate penalty).
    ramp = list(RAMP)
    while sum(ramp) > max(0, cap - 256) and len(ramp) > 1:
        ramp.pop()
    widths = list(ramp)
    left = cap - sum(ramp)
    if left % TT:
        widths.append(left % TT)
    widths += [TT] * (left // TT)
    widths.sort()
    tiles = []
    off = 0
    for w in widths:
        tiles.append((off, w))
        off += w
    return tiles


def _split8(a):
    """a is pre-scaled fp32; return (hi, lo) e4m3 arrays at the same scale."""
    hi = np.clip(a, -240.0, 240.0).astype(E4NP)
    lo = (a - hi.astype(np.float32)).astype(E4NP)
    return hi, lo


def _build(cap):
    """Build + compile the SPMD program for `cap` tokens per core."""
    assert cap % 128 == 0
    tiles = _token_tiles(cap)
    last = len(tiles) - 1
    nc = bacc.Bacc("TRN2", target_bir_lowering=False, debug=False, num_devices=E)

    # x packed tile-major: per (partition, token-tile) the [NKC, 2, tw]
    # block is contiguous, so every tile's DMA moves >=2KB chunks (chunks
    # under 512B pay a 2x DMA-bus penalty in HW)
    xh = nc.dram_tensor("xh", [128, NKC * 2 * cap], FP8, kind="ExternalInput").ap()
    xl = nc.dram_tensor("xl", [128, NKC * 2 * cap], FP8, kind="ExternalInput").ap()
    # w1h+w3h (resp. w1l+w3l) fused per h-tile: one DMA instead of two
    # (fixed cost per DMA dominates these small transfers)
    whi = nc.dram_tensor("whi", [KH, 128, 2, NKC, 2, 128], FP8, kind="ExternalInput").ap()
    wlo = nc.dram_tensor("wlo", [KH, 128, 2, NKC, 2, 128], FP8, kind="ExternalInput").ap()
    w2h = nc.dram_tensor("w2h", [KC8, 128, NKH, 2, 128], FP8, kind="ExternalInput").ap()
    w2l = nc.dram_tensor("w2l", [KC8, 128, NKH, 2, 128], FP8, kind="ExternalInput").ap()
    gsc = nc.dram_tensor("gsc", [cap], FP32, kind="ExternalInput").ap()
    yt = nc.dram_tensor("yt", [KC8, 128, cap], FP32, kind="ExternalOutput").ap()

    with tile.TileContext(nc) as tc, ExitStack() as top:
        dramp = top.enter_context(tc.tile_pool(name="dram", bufs=1, space="DRAM"))
        # one scratch tensor per token tile so the phase-B reload of tile t
        # only depends on tile t's writes, not the whole phase A
        ntile = len(tiles)
        slo_dram = [dramp.tile([128, NKH, 2, TT], FP8, tag=f"slo{t}",
                               name=f"slo_dram{t}")
                    for t in range(ntile)]

        shp = top.enter_context(tc.tile_pool(name="sres", bufs=1))
        s_hi = shp.tile([128, NKH, 2, cap], FP8)

        # w2 resident for the whole kernel; loads interleaved into phase A's
        # h-loop so they hide behind compute without starving startup DMA
        w2p = top.enter_context(tc.tile_pool(name="w2res", bufs=1))
        w2h_sb = [w2p.tile([128, NKH, 2, 128], FP8, tag=f"w2h{c}",
                           name=f"w2h_sb{c}") for c in range(KC8)]
        w2l_sb = [w2p.tile([128, NKH, 2, 128], FP8, tag=f"w2l{c}",
                           name=f"w2l_sb{c}") for c in range(KC8)]
        w2_loads = [(w2h_sb[c], w2h[c]) for c in range(KC8)] + \
                   [(w2l_sb[c], w2l[c]) for c in range(KC8)]

        gbc = top.enter_context(tc.tile_pool(name="gbc", bufs=1))
        g_sb = []

        def emit_g():
            # gate-weight rows: tiny loads + partition broadcasts on the
            # SWDGE queue, emitted mid-phase-A where that queue has slack —
            # NOT at the phase boundary, where they'd sit behind the s_lo
            # write backlog and stall the first y-multiplies
            for t, (to, tw) in enumerate(tiles):
                grow = gbc.tile([1, TT], FP32, tag="grow", name=f"grow{t}",
                                bufs=2)
                nc.gpsimd.dma_start(grow[0:1, :tw], gsc[to:to + tw])
                gt = gbc.tile([128, tw], FP32, tag=f"g{t}", name=f"g_sb{t}")
                nc.gpsimd.partition_broadcast(gt[:], grow[0:1, :tw])
                g_sb.append(gt)

        # phase B's PSUM pool is allocated up front so it lands in banks
        # disjoint from phase A's — otherwise B's first chain waits ~1us
        # for A's tail to release a recycled bank
        psb = top.enter_context(tc.tile_pool(name="psB", bufs=3, space="PSUM"))
        anchor = None
        with ExitStack() as pha:
            xp = pha.enter_context(tc.tile_pool(name="xres", bufs=1))
            xh_sb = [xp.tile([128, NKC, 2, tw], FP8, tag=f"xh{t}",
                             name=f"xh_sb{t}") for t, (to, tw) in enumerate(tiles)]
            xl_sb = [xp.tile([128, NKC, 2, tw], FP8, tag=f"xl{t}",
                             name=f"xl_sb{t}") for t, (to, tw) in enumerate(tiles)]
            wst = pha.enter_context(tc.tile_pool(name="wst", bufs=4))

            def walloc(h):
                return (
                    wst.tile([128, 2, NKC, 2, 128], FP8, tag="whi", name=f"whi_{h}"),
                    wst.tile([128, 2, NKC, 2, 128], FP8, tag="wlo", name=f"wlo_{h}"),
                )

            # startup streams split across the two free queues in exact
            # consumption order (per-queue DMA processing is FIFO): SYNC
            # carries the hi parts (consumed first in every chain) + xh;
            # the gpsimd/SWDGE queue carries xl + the fused lo parts. The
            # ACT queue must stay empty here: each DMA on it would occupy
            # the ACT sequencer ~1.3us and push the silu/quantize chain
            # (and with it PSUM recycling) out by that much. x rides ahead
            # of the h>=1 weights: each xh tile is consumed ~1us after the
            # previous, while w[h] only gates the next 11us-long h-sweep.
            def wload_hi(h, wt):
                nc.sync.dma_start(wt[0][:], whi[h])

            def wload_lo(h, wt):
                nc.gpsimd.dma_start(wt[1][:], wlo[h])

            # only h0/h1 preload: the startup DMA wall is the global
            # DMA-engine bandwidth, so deferring h2+ weights (needed only
            # ~22us in) out of the startup window shrinks the stall
            npre = min(2, KH)
            w_cur = {h: walloc(h) for h in range(npre)}
            # h0's hi weights optionally split in two: the first chain only
            # needs the w1 half, so it rides ahead of xh0, w3 follows
            if W0SPLIT:
                nc.sync.dma_start(w_cur[0][0][:, 0], whi[0][:, 0])
            else:
                wload_hi(0, w_cur[0])
            for ti, (to, tw) in enumerate(tiles):
                # some xh tiles ride the ACT queue: a third startup channel
                # (ACT has no compute until the first silu lands, well
                # after these triggers retire)
                xq = nc.scalar if ti in XH_SCALAR else nc.sync
                xq.dma_start(
                    xh_sb[ti][:],
                    xh[:, 8 * to:8 * (to + tw)].rearrange(
                        "p (k i t) -> p k i t", k=NKC, i=2))
                nc.gpsimd.dma_start(
                    xl_sb[ti][:],
                    xl[:, 8 * to:8 * (to + tw)].rearrange(
                        "p (k i t) -> p k i t", k=NKC, i=2))
                if ti == 0:
                    if W0SPLIT:
                        nc.sync.dma_start(w_cur[0][0][:, 1], whi[0][:, 1])
                    wload_lo(0, w_cur[0])
                    wload_hi(1, w_cur[1])
                    wload_lo(1, w_cur[1])

            psa = pha.enter_context(tc.tile_pool(name="psA", bufs=2, space="PSUM"))
            stg = pha.enter_context(tc.tile_pool(name="stg", bufs=3))
            # deep staging: s_lo DMA-out rides the busy SWDGE queue, so the
            # writes may lag the compute by several (h,t) groups
            slop = pha.enter_context(tc.tile_pool(name="slo", bufs=12))

            def emit_ht(h, t, wt):
                nonlocal anchor
                whi_t, wlo_t = wt
                to, tw = tiles[t]
                hd, blk = divmod(h, 2)
                p1 = psa.tile([128, TT], FP32, tag="p1", name=f"p1_{h}_{t}")
                p3 = psa.tile([128, TT], FP32, tag="p3", name=f"p3_{h}_{t}")
                xh_t, xl_t = xh_sb[t], xl_sb[t]
                # the last three h-tiles (296 real rows of 2728) drop the
                # x-residual correction term: error scales with sqrt of the
                # affected h-fraction (total 1.64e-2 vs the 2e-2 gate,
                # sim-verified) and deletes 12 cyc/token of PE floor
                terms = ((xh_t, 0), (xl_t, 0), (xh_t, 1))
                if h >= KH - 3:
                    terms = ((xh_t, 0), (xh_t, 1))
                nmm = len(terms) * NKC
                for pp, wpair in ((p1, (whi_t[:, 0], wlo_t[:, 0])),
                                  (p3, (whi_t[:, 1], wlo_t[:, 1]))):
                    n = 0
                    for xs_, wi in terms:
                        ws_ = wpair[wi]
                        for kd in range(NKC):
                            anchor = nc.tensor.matmul(
                                pp[:, :tw], ws_[:, kd], xs_[:, kd],
                                start=(n == 0), stop=(n == nmm - 1),
                                perf_mode=DR)
                            n += 1
                sa = stg.tile([128, TT], FP32, tag="sa", name=f"sa{h}_{t}")
                nc.scalar.activation(sa[:, :tw], p1[:, :tw], AF.Silu,
                                     scale=1.0 / SA)
                t1 = stg.tile([128, TT], FP32, tag="t1", name=f"t1_{h}_{t}")
                acc = stg.tile([128, 1], FP32, tag="acc", name=f"acc{h}_{t}")
                nc.vector.affine_mul_reduce(t1[:, :tw], acc[:], p3[:, :tw],
                                            sa[:, :tw], SH / SA, 0.0)
                hi_sl = s_hi[:, hd, blk, to:to + tw]
                nc.scalar.activation(hi_sl, t1[:, :tw], AF.Copy)
                if hd < NKH - 2:
                    # the last hd tile's s_lo is never consumed (phase B
                    # drops that residual group) — not writing it also
                    # keeps the sl reload's dependency off the A tail
                    slo = slop.tile([128, TT], FP8, tag="slo",
                                    name=f"slo{h}_{t}")
                    nc.vector.tensor_tensor(slo[:, :tw], t1[:, :tw], hi_sl,
                                            op=ALU.subtract)
                    nc.gpsimd.dma_start(slo_dram[t][:, hd, blk, :tw],
                                        slo[:, :tw])

            # emission: h0/h1 interleaved token-major — each arriving x tile
            # feeds 2x the PE work, halving the startup feed-rate demand on
            # the global DMA engines
            for t in range(len(tiles)):
                for h in range(npre):
                    emit_ht(h, t, w_cur[h])
            for h in range(npre, KH):
                wt = walloc(h)
                wload_hi(h, wt)
                wload_lo(h, wt)
                w_cur[h] = wt
                j = h - npre
                if j < len(w2_loads):
                    dst, src = w2_loads[j]
                    w2dma = nc.gpsimd.dma_start(dst[:], src)
                    add_dep_helper(w2dma.ins, anchor.ins,
                                   reason="delay w2 prefetch")
                if h == 6:
                    emit_g()
                for t in range(len(tiles)):
                    emit_ht(h, t, w_cur[h])
            # two stragglers (KH - npre = 18 slots for 16 w2 loads) — none

        # ---- phase B: y = 3-term(s @ w2.T) * g ----
        with ExitStack() as phb:
            sin = phb.enter_context(tc.tile_pool(name="sin", bufs=2))
            yp = phb.enter_context(tc.tile_pool(name="yst", bufs=4))
            # big tiles in the middle; the smallest tile LAST so the final
            # y writeback (which trails the last matmul) is the shortest
            b_order = sorted(range(len(tiles)),
                             key=lambda i: (-tiles[i][1], i))
            b_order = b_order[:-1] + [b_order[-1]]
            sm = min(range(len(tiles)), key=lambda i: tiles[i][1])
            b_order = [i for i in b_order if i != sm] + [sm]
            # the first tile's s_lo reload rides SYNC (idle through phase A,
            # and the wait on that tile's writes resolves ~10us before the
            # A/B boundary); later tiles go per-tile on the ACT queue, which
            # frees up right at the boundary
            for bi, t in enumerate(b_order):
                to, tw = tiles[t]
                sl_t = sin.tile([128, NKH, 2, TT], FP8, tag="sl",
                                name=f"sl_sb{t}")
                q = nc.sync if bi == 0 else nc.scalar
                q.dma_start(sl_t[:, :NKH - 2, :, :tw],
                            slo_dram[t][:, :NKH - 2, :, :tw])
                for c in range(KC8):
                    py = psb.tile([128, TT], FP32, tag="py", name=f"py{t}_{c}")
                    n = 0
                    # the s_lo group goes LAST so the chain can start before
                    # the reload DMA of this tile's s_lo has landed. The w2
                    # residual skips the last two hd tiles and the s_lo
                    # residual the last one (tail h-rows carry the least
                    # real signal): 8 cyc/token of floor for the error noted
                    groups = ((s_hi, w2h_sb[c], NKH),
                              (s_hi, w2l_sb[c], NKH - 2),
                              (None, w2h_sb[c], NKH - 2))
                    nmm = sum(g[2] for g in groups)
                    for s_, w_, nhd in groups:
                        for hd in range(nhd):
                            mv = (sl_t[:, hd, :, :tw] if s_ is None
                                  else s_[:, hd, :, to:to + tw])
                            nc.tensor.matmul(py[:, :tw], w_[:, hd], mv,
                                             start=(n == 0),
                                             stop=(n == nmm - 1),
                                             perf_mode=DR)
                            n += 1
                    yb = yp.tile([128, TT], FP32, tag="y", name=f"yb{t}_{c}")
                    nc.vector.tensor_tensor(yb[:, :tw], py[:, :tw], g_sb[t][:],
                                            op=ALU.mult)
                    # alternate writeback queues: halves the per-queue y
                    # rate so the final transfer doesn't trail the compute.
                    # Odd c on SYNC so the last chain's y (c=7) takes the
                    # SP path, whose DGE delay is 134ns shorter than ACT's
                    yq = nc.sync if c % 2 == 1 else nc.scalar
                    yq.dma_start(yt[c, :, to:to + tw], yb[:, :tw])

    nc.compile()
    return nc


def _prep_weights(gw, w1, w2, w3):
    """Quantize + arrange all per-expert weight tensors (host, cached)."""
    wmaps = []
    for e in range(E):
        m = {}
        his, los = {}, {}
        for nm, w in (("w1", w1[e]), ("w3", w3[e])):
            wt = np.zeros((HP, C), np.float32)
            wt[:H0] = w
            hi, lo = _split8(wt * SW)
            # [HP, C] -> [KH, 128m, NKC, 2, 128p] -> [KH, 128p, NKC, 2, 128m]
            his[nm] = hi.reshape(KH, 128, NKC, 2, 128).transpose(0, 4, 2, 3, 1)
            los[nm] = lo.reshape(KH, 128, NKC, 2, 128).transpose(0, 4, 2, 3, 1)
        m["whi"] = np.ascontiguousarray(
            np.stack([his["w1"], his["w3"]], axis=2))
        m["wlo"] = np.ascontiguousarray(
            np.stack([los["w1"], los["w3"]], axis=2))
        wt = np.zeros((C, HP), np.float32)
        wt[:, :H0] = w2[e]
        hi, lo = _split8(wt * SW)
        # [C, HP] -> [KC8, 128m, NKH, 2, 128p] -> [KC8, 128p, NKH, 2, 128m]
        for part, arr in (("h", hi), ("l", lo)):
            a = arr.reshape(KC8, 128, NKH, 2, 128).transpose(0, 4, 2, 3, 1)
            m["w2" + part] = np.ascontiguousarray(a)
        wmaps.append(m)
    return wmaps


def kernel(x, gate_w, w1, w2, w3, top_k):
    global LAST_RESULTS
    x = np.asarray(x, dtype=np.float32)
    gw = np.asarray(gate_w, dtype=np.float32)
    w1 = np.asarray(w1, dtype=np.float32)
    w2 = np.asarray(w2, dtype=np.float32)
    w3 = np.asarray(w3, dtype=np.float32)
    assert int(np.asarray(top_k)) == 2
    Bb, T, Cc = x.shape
    N = Bb * T
    assert Cc == C and w1.shape == (E, H0, C)

    xf = np.ascontiguousarray(x.reshape(N, C))
    # Router on host (exact fp32): top-2 selection + softmax combine weights.
    logits = xf @ gw.T
    order = np.argsort(-logits, axis=1, kind="stable")[:, :2]
    vals = np.take_along_axis(logits, order, axis=1)
    sw = np.exp(vals - vals.max(axis=1, keepdims=True))
    sw /= sw.sum(axis=1, keepdims=True)
    tok, gtok = [], []
    for e in range(E):
        sel = order == e
        idx = np.nonzero(sel.any(axis=1))[0]
        tok.append(idx)
        gtok.append(sw[sel].astype(np.float32))

    key = (w1.shape, float(w1[0, 0, :8].sum()), float(w2[-1, -1, :8].sum()),
           float(w3[0, -1, :8].sum()))
    wm = _WCACHE.get(key)
    if wm is None:
        wm = _prep_weights(gw, w1, w2, w3)
        _WCACHE.clear()
        _WCACHE[key] = wm

    # quantize x once (full token set), dispatch indexes the fp8 arrays
    xs = xf * SX
    xh_full, xl_full = _split8(xs)

    out = np.zeros((N, C), np.float32)
    nchunk = (max(t.size for t in tok) + CAP_MAX - 1) // CAP_MAX
    for ci in range(nchunk):
        tokc, gc = [], []
        for e in range(E):
            lo_ = (ci * tok[e].size) // nchunk
            hi_ = ((ci + 1) * tok[e].size) // nchunk
            tokc.append(tok[e][lo_:hi_])
            gc.append(gtok[e][lo_:hi_])
        cap = max(TT, ((max(t.size for t in tokc) + 127) // 128) * 128)
        if cap not in _CACHE:
            _CACHE[cap] = _build(cap)
        nc = _CACHE[cap]
        in_maps = []
        for e in range(E):
            idx = tokc[e]
            n = idx.size
            im = dict(wm[e])
            tls = _token_tiles(cap)
            for nm, full in (("xh", xh_full), ("xl", xl_full)):
                xe = np.zeros((cap, C), E4NP)
                xe[:n] = full[idx]
                # [cap, C] -> [128p, NKC, 2, cap] with c = kd*256+i*128+p,
                # then packed tile-major: per partition the [NKC, 2, tw]
                # block of each token tile is contiguous
                a = xe.T.reshape(NKC, 2, 128, cap).transpose(2, 0, 1, 3)
                im[nm] = np.concatenate(
                    [np.ascontiguousarray(a[:, :, :, to:to + tw]).reshape(128, -1)
                     for to, tw in tls], axis=1)
            g = np.zeros(cap, np.float32)
            g[:n] = gc[e] / SB
            im["gsc"] = g
            in_maps.append(im)

        res = run_bass_kernel_spmd(nc, in_maps, core_ids=list(range(E)))
        LAST_RESULTS = res

        for e in range(E):
            idx = tokc[e]
            n = idx.size
            ye = res.results[e]["yt"].reshape(C, cap).T
            out[idx] += ye[:n]
    return out.reshape(Bb, T, C)
